# revision 1
# baseline (speedup 1.0000x reference)
"""Trainium2 Bass kernel for a dense transformer block (B=4, N=2048, C=768, H=12).

Sharding: 8 cores = 4 batches x 2 sequence halves. Each core receives its
batch's rows rolled so its own 1024 query rows are rows 0:1023 (softmax is
permutation-invariant over keys, so key order doesn't matter). Each core
computes LN1 over all 2048 rows, K/V per 4-head group and attention + MLP for
its own 1024 rows, returning a [1024, 768] output slice. No collectives.

All matmuls run in float32r (full PE rate, ~1e-4 rounding). Dataflow stays in
transposed [channel, token] layouts so contractions land on SBUF partitions.
Softmax denominators ride the values matmul as an appended ones-column; the
per-query 1/denom is applied by folding diag(r) into the PE transpose that
restores each head's [channel, token] layout. SBUF is managed as tag-chained
slots (five 24KB/partition slots rotate through the phase-chained tensors).
"""

import numpy as np

B, N, C = 4, 2048, 768
H, DH = 12, 64
HID = 4 * C
SCALE = DH ** -0.5
EPS = 1e-5

P = 128
CT = C // P          # 6
NT = N // P          # 16
NO = N // 2          # 1024 own rows
NOT_ = NO // P       # 8
HT = HID // P        # 24


def _build_bass():
    import concourse.bass as bass
    import concourse.tile as tile
    from concourse import bacc, mybir
    from concourse.masks import make_identity
    from concourse.alu_op_type import AluOpType as A

    F32 = mybir.dt.float32
    F32R = mybir.dt.float32r
    AF = mybir.ActivationFunctionType

    nc = bacc.Bacc("TRN2", target_bir_lowering=False, num_swdge_queues=4)

    xb = nc.dram_tensor("xb", [N, C], F32, kind="ExternalInput")
    w_qkv = nc.dram_tensor("w_qkv", [C, 3 * C], F32, kind="ExternalInput")
    w_proj = nc.dram_tensor("w_proj", [C, C], F32, kind="ExternalInput")
    w_fc1 = nc.dram_tensor("w_fc1", [C, HID], F32, kind="ExternalInput")
    w_fc2 = nc.dram_tensor("w_fc2", [HID, C], F32, kind="ExternalInput")
    ln1_g = nc.dram_tensor("ln1_g", [C], F32, kind="ExternalInput")
    ln1_b = nc.dram_tensor("ln1_b", [C], F32, kind="ExternalInput")
    ln2_g = nc.dram_tensor("ln2_g", [C], F32, kind="ExternalInput")
    ln2_b = nc.dram_tensor("ln2_b", [C], F32, kind="ExternalInput")
    b_proj = nc.dram_tensor("b_proj", [C], F32, kind="ExternalInput")
    b_fc1 = nc.dram_tensor("b_fc1", [HID], F32, kind="ExternalInput")
    b_fc2 = nc.dram_tensor("b_fc2", [C], F32, kind="ExternalInput")
    out = nc.dram_tensor("out", [NO, C], F32, kind="ExternalOutput")

    dma = nc.gpsimd.dma_start

    with tile.TileContext(nc) as tc:
        consts = tc.alloc_tile_pool(name="consts", bufs=1)
        pbc = tc.alloc_tile_pool(name="pbc", bufs=1)        # LN gamma/beta bcast
        psmall = tc.alloc_tile_pool(name="psmall", bufs=1)  # denominators etc.
        work = tc.alloc_tile_pool(name="work", bufs=2)
        main = tc.alloc_tile_pool(name="main", bufs=1)      # five 24KB slots
        stream = tc.alloc_tile_pool(name="stream", bufs=2)  # weights/exp stream
        pkt = tc.alloc_tile_pool(name="pkt", bufs=1)        # K^T per pair

        ident = consts.tile([P, P], F32)
        make_identity(nc, ident)
        ident_r = consts.tile([P, P], F32R)
        nc.vector.tensor_copy(ident_r, ident)
        eps_t = consts.tile([P, 1], F32)
        nc.vector.memset(eps_t, EPS)
        ones_col = consts.tile([P, 1], F32)
        nc.vector.memset(ones_col, 1.0)
        bpT = consts.tile([P, CT], F32)
        dma(out=bpT, in_=b_proj[:].rearrange("(t p) -> p t", p=P))
        bf1T = consts.tile([P, HT], F32)
        dma(out=bf1T, in_=b_fc1[:].rearrange("(t p) -> p t", p=P))
        bf2T = consts.tile([P, CT], F32)
        dma(out=bf2T, in_=b_fc2[:].rearrange("(t p) -> p t", p=P))

        def layernorm_tile(x_t, g_bc, b_bc):
            st = work.tile([P, 3, 6], F32, tag="ln_st")
            for s in range(3):
                nc.vector.bn_stats(out=st[:, s, :], in_=x_t[:, s * 256:(s + 1) * 256])
            mv = work.tile([P, 2], F32, tag="ln_mv")
            nc.vector.bn_aggr(out=mv, in_=st)
            lnv = work.tile([P, 1], F32, tag="ln_lnv")
            nc.scalar.activation(out=lnv, in_=mv[:, 1:2], func=AF.Ln, bias=eps_t)
            r = work.tile([P, 1], F32, tag="ln_r")
            nc.scalar.activation(out=r, in_=lnv, func=AF.Exp, scale=-0.5)
            h = work.tile([P, C], F32, tag="ln_h")
            nc.vector.tensor_scalar(out=h, in0=x_t, scalar1=mv[:, 0:1], scalar2=r,
                                    op0=A.subtract, op1=A.mult)
            nc.vector.tensor_tensor(out=h, in0=h, in1=g_bc, op=A.mult)
            nc.vector.tensor_tensor(out=h, in0=h, in1=b_bc, op=A.add)
            return h

        def transpose_768(src, dst_view, ps_pool, ps_tag="tr"):
            tp = ps_pool.tile([P, C], F32, tag=ps_tag)
            for t in range(CT):
                nc.tensor.transpose(tp[:, t * P:(t + 1) * P],
                                    src[:, t * P:(t + 1) * P], ident)
            nc.vector.tensor_copy(out=dst_view,
                                  in_=tp[:].rearrange("p (t n) -> p t n", t=CT))

        # ---------------- Phase A: LN1 + transpose -> hT0/hT1 [128, 3, 2048] f32r
        hT0 = main.tile([P, 3, N], F32R, tag="S1")
        hT1 = main.tile([P, 3, N], F32R, tag="S2")

        def hts(kt, sl):
            return hT0[:, kt, sl] if kt < 3 else hT1[:, kt - 3, sl]

        g1_bc = pbc.tile([P, C], F32, tag="g_bc")
        dma(out=g1_bc, in_=ln1_g[:].partition_broadcast(P))
        b1_bc = pbc.tile([P, C], F32, tag="b_bc")
        dma(out=b1_bc, in_=ln1_b[:].partition_broadcast(P))
        with tc.tile_pool(name="ps_trA", bufs=2, space="PSUM") as ps_trA:
            for i in range(NT):
                x_t = work.tile([P, C], F32, tag="io")
                dma(out=x_t, in_=xb[i * P:(i + 1) * P, :])
                hg = layernorm_tile(x_t, g1_bc, b1_bc)
                tp = ps_trA.tile([P, C], F32, tag="tr")
                for t in range(CT):
                    nc.tensor.transpose(tp[:, t * P:(t + 1) * P],
                                        hg[:, t * P:(t + 1) * P], ident)
                nc.vector.tensor_copy(
                    out=hT0[:, :, i * P:(i + 1) * P],
                    in_=tp[:, 0:384].rearrange("p (t n) -> p t n", t=3))
                nc.vector.tensor_copy(
                    out=hT1[:, :, i * P:(i + 1) * P],
                    in_=tp[:, 384:768].rearrange("p (t n) -> p t n", t=3))

        # ---------------- Phase B: attention, per group of 4 heads (2 pairs)
        YTraw = main.tile([P, CT, NO], F32, tag="S3")
        den = psmall.tile([H, NO], F32)
        with tc.tile_pool(name="ps_b", bufs=1, space="PSUM") as ps_b, \
             tc.tile_pool(name="ps_y", bufs=1, space="PSUM") as ps_y:
            for pg in range(3):
                # V for heads 4pg..4pg+3, token-major with an appended ones col
                V_g = main.tile([P, NT, 4 * 65], F32R, tag="S5")
                wv = stream.tile([P, CT, 256], F32R, tag="w")
                dma(out=wv, in_=w_qkv[:, 2 * C + 256 * pg:2 * C + 256 * (pg + 1)]
                    .rearrange("(t p) j -> p t j", p=P))
                for i in range(NT):
                    vps = ps_b.tile([P, 256], F32, tag="sA" if i % 2 == 0 else "sB")
                    for kt in range(CT):
                        nc.tensor.matmul(vps, hts(kt, slice(i * P, (i + 1) * P)),
                                         wv[:, kt, :],
                                         start=(kt == 0), stop=(kt == CT - 1))
                    vv = V_g[:, i, :].rearrange("p (h d) -> p h d", h=4)
                    nc.vector.tensor_copy(
                        out=vv[:, :, 0:64],
                        in_=vps[:].rearrange("p (h d) -> p h d", h=4))
                    nc.vector.tensor_copy(out=vv[:, :, 64:65],
                                          in_=ones_col.to_broadcast((P, 4, 1)))
                for pr in range(2):
                    hp = 2 * pg + pr
                    # Q^T (own rows) / K^T (all rows) for this head pair
                    wq = stream.tile([P, CT, P], F32R, tag="w")
                    dma(out=wq, in_=w_qkv[:, hp * P:(hp + 1) * P]
                        .rearrange("(t p) j -> p t j", p=P))
                    qps = ps_b.tile([P, NO], F32, tag="sA")
                    for ch in range(2):
                        for kt in range(CT):
                            nc.tensor.matmul(qps[:, ch * 512:(ch + 1) * 512],
                                             wq[:, kt, :],
                                             hts(kt, slice(ch * 512, (ch + 1) * 512)),
                                             start=(kt == 0), stop=(kt == CT - 1))
                    QT = stream.tile([P, NO], F32R, tag="qt")
                    nc.vector.tensor_copy(QT, qps)
                    wk = stream.tile([P, CT, P], F32R, tag="w")
                    dma(out=wk, in_=w_qkv[:, C + hp * P:C + (hp + 1) * P]
                        .rearrange("(t p) j -> p t j", p=P))
                    KT = pkt.tile([P, N], F32R, tag="kt")
                    for half in range(2):
                        kps = ps_b.tile([P, NO], F32, tag="sA" if half == 0 else "sB")
                        for ch in range(2):
                            c0 = half * NO + ch * 512
                            for kt in range(CT):
                                nc.tensor.matmul(kps[:, ch * 512:(ch + 1) * 512],
                                                 wk[:, kt, :],
                                                 hts(kt, slice(c0, c0 + 512)),
                                                 start=(kt == 0), stop=(kt == CT - 1))
                        nc.vector.tensor_copy(KT[:, half * NO:(half + 1) * NO], kps)

                    yA = ps_y.tile([65, NO], F32, tag="yA")
                    yB = ps_y.tile([65, NO], F32, tag="yB")
                    for m in range(NT):
                        # separate per-head score tiles (separate PSUM banks ->
                        # the two row-group matmuls run concurrently, and exp of
                        # head A overlaps the QK matmuls of head B / tile m+1)
                        spsA = ps_b.tile([P, NO], F32, tag="sA")
                        spsB = ps_b.tile([P, NO], F32, tag="sB")
                        for ch in range(2):
                            nc.tensor.matmul(spsA[:, ch * 512:(ch + 1) * 512],
                                             KT[0:64, m * P:(m + 1) * P],
                                             QT[0:64, ch * 512:(ch + 1) * 512],
                                             start=True, stop=True,
                                             tile_position=(0, 0))
                        for ch in range(2):
                            nc.tensor.matmul(spsB[:, ch * 512:(ch + 1) * 512],
                                             KT[64:128, m * P:(m + 1) * P],
                                             QT[64:128, ch * 512:(ch + 1) * 512],
                                             start=True, stop=True,
                                             tile_position=(64, 0))
                        eA = stream.tile([P, NO], F32R, tag="e")
                        nc.scalar.activation(out=eA, in_=spsA[:], func=AF.Exp,
                                             scale=SCALE)
                        eB = stream.tile([P, NO], F32R, tag="e")
                        nc.scalar.activation(out=eB, in_=spsB[:],
                                             func=AF.Exp, scale=SCALE)
                        for ch in range(2):
                            nc.tensor.matmul(yA[:, ch * 512:(ch + 1) * 512],
                                             V_g[:, m, 65 * 2 * pr:65 * 2 * pr + 65],
                                             eA[:, ch * 512:(ch + 1) * 512],
                                             start=(m == 0), stop=(m == NT - 1))
                        for ch in range(2):
                            nc.tensor.matmul(yB[:, ch * 512:(ch + 1) * 512],
                                             V_g[:, m, 65 * (2 * pr + 1):65 * (2 * pr + 1) + 65],
                                             eB[:, ch * 512:(ch + 1) * 512],
                                             start=(m == 0), stop=(m == NT - 1))
                    # psum -> sbuf; odd head + denominators shift partitions by DMA
                    ytA = stream.tile([65, NO], F32, tag="e")
                    ytB = stream.tile([65, NO], F32, tag="e")
                    nc.vector.tensor_copy(out=YTraw[0:64, hp, :], in_=yA[0:64, :])
                    nc.vector.tensor_copy(out=ytA[64:65, :], in_=yA[64:65, :])
                    nc.vector.tensor_copy(ytB, yB)
                    dma(out=YTraw[64:128, hp, :], in_=ytB[0:64, :])
                    dma(out=den[2 * hp:2 * hp + 1, :], in_=ytA[64:65, :])
                    dma(out=den[2 * hp + 1:2 * hp + 2, :], in_=ytB[64:65, :])

        # ---------------- Phase C: normalize y by 1/den via diag-scaled transposes
        YTn = main.tile([P, CT, NO], F32R, tag="S4")
        rinv = psmall.tile([H, NO], F32R)
        with nc.allow_low_precision(reason="fp32r rounding of softmax denom"):
            nc.vector.reciprocal(out=rinv, in_=den)
        rT = psmall.tile([P, NOT_, H], F32)
        with tc.tile_pool(name="ps_n", bufs=3, space="PSUM") as ps_n:
            for i in range(NOT_):
                rtp = ps_n.tile([P, H], F32, tag="rT", bufs=2)
                nc.tensor.matmul(rtp, rinv[:, i * P:(i + 1) * P], ident_r[0:H, 0:H],
                                 start=True, stop=True)
                nc.vector.tensor_copy(out=rT[:, i, :], in_=rtp)
            for hp in range(CT):
                for i in range(NOT_):
                    # both heads of the pair -> y [token, dim] with tokens on
                    # partitions; 1/den rides the copy as a per-partition scalar
                    ysb2 = work.tile([P, P], F32, tag="ysb2")
                    for sub in range(2):
                        h, lo = 2 * hp + sub, 64 * sub
                        yps = ps_n.tile([P, 64], F32, tag="y_nt")
                        nc.tensor.transpose(yps,
                                            YTraw[lo:lo + 64, hp, i * P:(i + 1) * P],
                                            ident[lo:lo + 64, lo:lo + 64])
                        nc.vector.tensor_scalar(out=ysb2[:, lo:lo + 64], in0=yps,
                                                scalar1=rT[:, i, h:h + 1],
                                                scalar2=None, op0=A.mult)
                    ytp = ps_n.tile([P, P], F32, tag="yT_n")
                    nc.tensor.transpose(ytp, ysb2, ident)
                    nc.vector.tensor_copy(out=YTn[:, hp, i * P:(i + 1) * P],
                                          in_=ytp)

        # ---------------- Phase D: proj -> attnT (S5 slot)
        attnT = main.tile([P, CT, NO], F32, tag="S5")
        with tc.tile_pool(name="ps_p", bufs=4, space="PSUM") as ps_p:
            for cp in range(CT):
                wp = stream.tile([P, CT, P], F32R, tag="w")
                dma(out=wp, in_=w_proj[:, cp * P:(cp + 1) * P]
                    .rearrange("(t p) j -> p t j", p=P))
                pps = ps_p.tile([P, NO], F32, tag="p")
                for ch in range(2):
                    for kt in range(CT):
                        nc.tensor.matmul(pps[:, ch * 512:(ch + 1) * 512],
                                         wp[:, kt, :],
                                         YTn[:, kt, ch * 512:(ch + 1) * 512],
                                         start=(kt == 0), stop=(kt == CT - 1))
                nc.vector.tensor_scalar(out=attnT[:, cp, :], in0=pps,
                                        scalar1=bpT[:, cp:cp + 1], scalar2=None,
                                        op0=A.add)

        # ---------------- Phase E: attn + residual -> x2; LN2 -> x2lnT
        x2 = main.tile([P, NOT_, C], F32, tag="S4")
        x2lnT = main.tile([P, CT, NO], F32R, tag="S3")
        g2_bc = pbc.tile([P, C], F32, tag="g_bc")
        dma(out=g2_bc, in_=ln2_g[:].partition_broadcast(P))
        b2_bc = pbc.tile([P, C], F32, tag="b_bc")
        dma(out=b2_bc, in_=ln2_b[:].partition_broadcast(P))
        with tc.tile_pool(name="ps_trE", bufs=4, space="PSUM") as ps_trE:
            # sweep 1: attn^T -> attn, + residual -> x2 (PE + DVE pipeline)
            for i in range(NOT_):
                tp = ps_trE.tile([P, C], F32, tag="tr")
                for t in range(CT):
                    nc.tensor.transpose(tp[:, t * P:(t + 1) * P],
                                        attnT[:, t, i * P:(i + 1) * P], ident)
                xo = work.tile([P, C], F32, tag="io")
                dma(out=xo, in_=xb[i * P:(i + 1) * P, :])
                nc.vector.tensor_tensor(out=x2[:, i, :], in0=tp, in1=xo, op=A.add)
            # sweep 2: LN2 + transpose -> x2lnT
            for i in range(NOT_):
                hg2 = layernorm_tile(x2[:, i, :], g2_bc, b2_bc)
                transpose_768(hg2, x2lnT[:, :, i * P:(i + 1) * P], ps_trE)

        # ---------------- Phase F: MLP + residual + output, per 512-token half.
        # fc2 accumulates into six persistent PSUM banks as each gelu tile is
        # produced, so fc1/gelu/fc2 fully pipeline and no activation buffer is
        # needed in SBUF. w_fc2 row-slices load in natural [hid, c'] layout.
        for nh in range(2):
            sl = slice(nh * 512, (nh + 1) * 512)
            with tc.tile_pool(name="ps_mA%d" % nh, bufs=1, space="PSUM") as ps_mA:
                f2s = [ps_mA.tile([P, 512], F32, tag="f2c%d" % cp,
                                  name="f2acc%d_%d" % (nh, cp))
                       for cp in range(CT)]
                for ht in range(HT):
                    w1 = stream.tile([P, CT, P], F32R, tag="wf1", bufs=2)
                    dma(out=w1, in_=w_fc1[:, ht * P:(ht + 1) * P]
                        .rearrange("(t p) j -> p t j", p=P))
                    w2r = stream.tile([P, C], F32R, tag="wf2", bufs=2)
                    dma(out=w2r, in_=w_fc2[ht * P:(ht + 1) * P, :])
                    fps = ps_mA.tile([P, 512], F32,
                                     tag="f1a" if ht % 2 == 0 else "f1b")
                    for kt in range(CT):
                        nc.tensor.matmul(fps, w1[:, kt, :], x2lnT[:, kt, sl],
                                         start=(kt == 0), stop=(kt == CT - 1))
                    ga = work.tile([P, 512], F32R, tag="ga", bufs=3)
                    nc.scalar.activation(out=ga, in_=fps[:], func=AF.Gelu,
                                         bias=bf1T[:, ht:ht + 1])
                    for cp in range(CT):
                        nc.tensor.matmul(f2s[cp], w2r[:, cp * P:(cp + 1) * P], ga,
                                         start=(ht == 0), stop=(ht == HT - 1))
                mlpT = main.tile([P, CT, 512], F32, tag="S5")
                for cp in range(CT):
                    nc.vector.tensor_scalar(out=mlpT[:, cp, :], in0=f2s[cp],
                                            scalar1=bf2T[:, cp:cp + 1],
                                            scalar2=None, op0=A.add)
            with tc.tile_pool(name="ps_o%d" % nh, bufs=2, space="PSUM") as ps_o:
                for i in range(4):
                    it = nh * 4 + i
                    tp = ps_o.tile([P, C], F32, tag="tr")
                    for t in range(CT):
                        nc.tensor.transpose(tp[:, t * P:(t + 1) * P],
                                            mlpT[:, t, i * P:(i + 1) * P], ident)
                    o_sb = work.tile([P, C], F32, tag="io")
                    nc.vector.tensor_tensor(out=o_sb, in0=tp, in1=x2[:, it, :],
                                            op=A.add)
                    dma(out=out[it * P:(it + 1) * P, :], in_=o_sb)

        pkt.release()
        stream.release()
        main.release()
        work.release()
        psmall.release()
        pbc.release()
        consts.release()

    nc.compile()
    return nc


_NC_CACHE = None


def kernel(x, ln1_g, ln1_b, w_qkv, w_proj, b_proj, ln2_g, ln2_b,
           w_fc1, b_fc1, w_fc2, b_fc2):
    global _NC_CACHE
    from concourse.bass_utils import run_bass_kernel_spmd

    x = np.asarray(x, dtype=np.float32)
    shared = {
        "w_qkv": np.asarray(w_qkv, np.float32),
        "w_proj": np.asarray(w_proj, np.float32),
        "w_fc1": np.asarray(w_fc1, np.float32),
        "w_fc2": np.asarray(w_fc2, np.float32),
        "ln1_g": np.asarray(ln1_g, np.float32),
        "ln1_b": np.asarray(ln1_b, np.float32),
        "ln2_g": np.asarray(ln2_g, np.float32),
        "ln2_b": np.asarray(ln2_b, np.float32),
        "b_proj": np.asarray(b_proj, np.float32),
        "b_fc1": np.asarray(b_fc1, np.float32),
        "b_fc2": np.asarray(b_fc2, np.float32),
    }
    in_maps = []
    for c in range(8):
        b, h = c // 2, c % 2
        xbv = np.ascontiguousarray(np.roll(x[b], -h * NO, axis=0))
        in_maps.append({"xb": xbv, **shared})

    if _NC_CACHE is None:
        _NC_CACHE = _build_bass()
    res = run_bass_kernel_spmd(_NC_CACHE, in_maps, core_ids=list(range(8)))

    outp = np.empty((B, N, C), np.float32)
    for c in range(8):
        b, h = c // 2, c % 2
        outp[b, h * NO:(h + 1) * NO, :] = res.results[c]["out"]
    return outp



# revision 7
# speedup vs baseline: 1.4569x; 1.4569x over previous
"""Trainium2 Bass kernel for a dense transformer block (B=4, N=2048, C=768, H=12).

Sharding: 8 cores = 4 batches x 2 sequence halves (queries split; K/V duplicated
per batch pair, no collectives). Each core receives its batch rolled so its own
1024 query rows are rows 0:1023.

v2 dataflow (cost-model-driven):
- All projection/attention-value/MLP matmuls run in fp8e4m3 with DoubleRow perf
  mode (2 contraction k-tiles per instruction, 0.5 cycles/column = 4x fp32r).
  Weights are folded (LN gains), prescaled by 32 on the host, and cast to fp8;
  the 1/32 unscale + bias ride the PSUM->SBUF copies (or gelu's scale/bias).
- Scores stay bf16 (precision-sensitive); softmax exp runs on the Act engine in
  [128, 4, 512] PSUM tiles (2048-column calls) writing fp8 directly; the
  denominator rides the value matmul as a 65th ones-row of V.
- Per-query 1/den is applied via a PE ones-outer-product broadcast plus one DVE
  multiply per (head, chunk) - no transposes.
- LN uses Sqrt+DVE-reciprocal (no Ln/Exp table thrash); only 4 act-table loads.
- All DMA goes through SP HWDGE (Pool engine stays free for psum->sbuf copies).
"""

import numpy as np

B, N, C = 4, 2048, 768
H, DH = 12, 64
HID = 4 * C
SCALE = DH ** -0.5
EPS = 1e-5
WS = 32.0
IWS = 1.0 / WS

P = 128
CT = C // P          # 6
NT = N // P          # 16
NO = N // 2          # 1024 own rows
NOT_ = NO // P       # 8
HT = HID // P        # 24


def _build_bass():
    import concourse.bass as bass
    import concourse.tile as tile
    from concourse import bacc, mybir
    from concourse.masks import make_identity
    from concourse.alu_op_type import AluOpType as A

    F32 = mybir.dt.float32
    BF = mybir.dt.bfloat16
    F8 = mybir.dt.float8e4
    AF = mybir.ActivationFunctionType
    DR = mybir.MatmulPerfMode.DoubleRow

    nc = bacc.Bacc("TRN2", target_bir_lowering=False, num_swdge_queues=4)

    xb = nc.dram_tensor("xb", [N, C], BF, kind="ExternalInput")
    w_qkv8 = nc.dram_tensor("w_qkv8", [C, 3 * C], F8, kind="ExternalInput")
    w_proj8 = nc.dram_tensor("w_proj8", [DH, H * C], F8, kind="ExternalInput")
    w_fc18 = nc.dram_tensor("w_fc18", [C, HID], F8, kind="ExternalInput")
    w_fc28 = nc.dram_tensor("w_fc28", [HID, C], F8, kind="ExternalInput")
    qkv_bias = nc.dram_tensor("qkv_bias", [3 * C], F32, kind="ExternalInput")
    fc1_bias = nc.dram_tensor("fc1_bias", [HID], F32, kind="ExternalInput")
    b_proj_bf = nc.dram_tensor("b_proj_bf", [C], BF, kind="ExternalInput")
    b_fc2 = nc.dram_tensor("b_fc2", [C], F32, kind="ExternalInput")
    out = nc.dram_tensor("out", [NO, C], F32, kind="ExternalOutput")

    dma = nc.sync.dma_start

    with tile.TileContext(nc) as tc:
        consts = tc.alloc_tile_pool(name="consts", bufs=1)
        wpool = tc.alloc_tile_pool(name="wpool", bufs=1)
        big = tc.alloc_tile_pool(name="big", bufs=1)
        qkpool = tc.alloc_tile_pool(name="qkpool", bufs=2)
        work = tc.alloc_tile_pool(name="work", bufs=2)
        io = tc.alloc_tile_pool(name="io", bufs=4)
        estream = tc.alloc_tile_pool(name="estream", bufs=2)

        ident_bf = consts.tile([P, P], BF)
        make_identity(nc, ident_bf)
        ones_bf = consts.tile([P, DH], BF)
        nc.gpsimd.memset(ones_bf, 1.0)
        eps_t = consts.tile([P, 1], F32)
        nc.vector.memset(eps_t, EPS)
        qbT = consts.tile([P, 18], F32)
        dma(out=qbT, in_=qkv_bias[:].rearrange("(t p) -> p t", p=P))
        f1bT = consts.tile([P, HT], F32)
        dma(out=f1bT, in_=fc1_bias[:].rearrange("(t p) -> p t", p=P))
        bf2T = consts.tile([P, CT], F32)
        dma(out=bf2T, in_=b_fc2[:].rearrange("(t p) -> p t", p=P))
        vbias_bc = consts.tile([P, C], F32)
        dma(out=vbias_bc, in_=qkv_bias[2 * C:].partition_broadcast(P))
        bproj_bc = consts.tile([P, C], BF)
        dma(out=bproj_bc, in_=b_proj_bf[:].partition_broadcast(P))

        Wqkv_s = wpool.tile([P, CT, 3 * C], F8)
        Wp_s = wpool.tile([DH, H, C], F8)
        Wfc1_s = wpool.tile([P, CT, HID], F8)
        Wfc2_s = wpool.tile([P, HT, C], F8)

        hT = big.tile([P, CT, N], F8)          # LN1(x)^T, fp8
        VW = 80  # V cols + ones + pad: dual-fp8 ldweights needs M % 16 == 0
        Vf8 = big.tile([P, NT, H, VW], F8)
        Yf8 = big.tile([DH, H, NO], F8)         # normalized y, proj rhs
        x2lnT = big.tile([P, CT, NO], F8)       # LN2(x2)^T
        GA8 = big.tile([P, HT, 512], F8)        # gelu acts, per token-half
        x2 = big.tile([P, NOT_, C], BF)         # x + attn + b_proj

        nc.vector.memset(Vf8[:, :, :, DH:VW], 1.0)

        def ln_apply(src, dst_bf, csize):
            # dst = (src - mean) * rsqrt(var+eps); gains/biases are host-folded
            st = work.tile([P, 3, 6], F32, tag="ln_st")
            for s in range(3):
                nc.vector.bn_stats(out=st[:, s, :], in_=src[:, s * 256:(s + 1) * 256])
            mv = work.tile([P, 2], F32, tag="ln_mv")
            nc.vector.bn_aggr(out=mv, in_=st)
            sd = work.tile([P, 1], F32, tag="ln_sd")
            nc.scalar.activation(out=sd, in_=mv[:, 1:2], func=AF.Sqrt, bias=eps_t)
            r = work.tile([P, 1], F32, tag="ln_r")
            nc.vector.reciprocal(out=r, in_=sd)
            nc.vector.tensor_scalar(out=dst_bf, in0=src, scalar1=mv[:, 0:1],
                                    scalar2=r, op0=A.subtract, op1=A.mult)

        # ---------------- Phase A: LN1 -> hT (fp8, transposed)
        with tc.tile_pool(name="ps_a", bufs=3, space="PSUM") as ps_a:
            for i in range(NT):
                x_t = io.tile([P, C], BF, tag="x")
                dma(out=x_t, in_=xb[i * P:(i + 1) * P, :])
                if i == 7:
                    dma(out=Wqkv_s,
                        in_=w_qkv8[:].rearrange("(t p) j -> p t j", p=P))
                z = work.tile([P, C], BF, tag="z")
                ln_apply(x_t, z, C)
                tp = ps_a.tile([P, C], BF, tag="tr")
                for t in range(CT):
                    nc.tensor.transpose(tp[:, t * P:(t + 1) * P],
                                        z[:, t * P:(t + 1) * P], ident_bf)
                nc.vector.tensor_copy(
                    out=hT[:, :, i * P:(i + 1) * P],
                    in_=tp[:].rearrange("p (t n) -> p t n", t=CT))
        dma(out=Wp_s, in_=w_proj8[:].rearrange("d (h j) -> d h j", h=H))
        dma(out=Wfc1_s, in_=w_fc18[:].rearrange("(t p) j -> p t j", p=P))
        dma(out=Wfc2_s, in_=w_fc28[:].rearrange("(t p) j -> p t j", p=P))

        # ---------------- Phase B0: V (token-major) for all heads
        with tc.tile_pool(name="ps_v", bufs=2, space="PSUM") as ps_v:
            for i in range(NT):
                vps = ps_v.tile([P, C], F32, tag="v")
                for k2 in range(3):
                    for sl, c0 in ((512, 0), (256, 512)):
                        nc.tensor.matmul(
                            vps[:, c0:c0 + sl],
                            hT[:, 2 * k2:2 * k2 + 2, i * P:(i + 1) * P],
                            Wqkv_s[:, 2 * k2:2 * k2 + 2, 2 * C + c0:2 * C + c0 + sl],
                            start=(k2 == 0), stop=(k2 == 2), perf_mode=DR)
                nc.vector.scalar_tensor_tensor(
                    out=Vf8[:, i, :, 0:DH],
                    in0=vps[:].rearrange("p (h d) -> p h d", h=H),
                    scalar=IWS, in1=vbias_bc[:].rearrange("p (h d) -> p h d", h=H),
                    op0=A.mult, op1=A.add)

        # ---------------- Phase B: attention, head-sequential
        with tc.tile_pool(name="ps_b", bufs=1, space="PSUM") as ps_b:
            for h in range(H):
                hp, sub = h // 2, h % 2
                base = sub * DH
                if sub == 0:
                    # Q (own rows) and K (all rows) for this head pair
                    QT_s = qkpool.tile([P, NO], BF, tag="qt")
                    for q2 in range(2):
                        qps = ps_b.tile([P, 512], F32, tag="qk")
                        for k2 in range(3):
                            nc.tensor.matmul(
                                qps,
                                Wqkv_s[:, 2 * k2:2 * k2 + 2, hp * P:(hp + 1) * P],
                                hT[:, 2 * k2:2 * k2 + 2, q2 * 512:(q2 + 1) * 512],
                                start=(k2 == 0), stop=(k2 == 2), perf_mode=DR)
                        nc.vector.tensor_scalar(
                            out=QT_s[:, q2 * 512:(q2 + 1) * 512], in0=qps,
                            scalar1=IWS, scalar2=qbT[:, hp:hp + 1],
                            op0=A.mult, op1=A.add)
                    KT_s = qkpool.tile([P, N], BF, tag="kt")
                    for q4 in range(4):
                        kps = ps_b.tile([P, 512], F32, tag="qk")
                        for k2 in range(3):
                            nc.tensor.matmul(
                                kps,
                                Wqkv_s[:, 2 * k2:2 * k2 + 2, C + hp * P:C + (hp + 1) * P],
                                hT[:, 2 * k2:2 * k2 + 2, q4 * 512:(q4 + 1) * 512],
                                start=(k2 == 0), stop=(k2 == 2), perf_mode=DR)
                        nc.vector.tensor_scalar(
                            out=KT_s[:, q4 * 512:(q4 + 1) * 512], in0=kps,
                            scalar1=IWS, scalar2=qbT[:, 6 + hp:7 + hp],
                            op0=A.mult, op1=A.add)
                for ch in range(2):
                    y = ps_b.tile([80, 512], F32, tag="y", bufs=2)
                    for g in range(4):
                        S = ps_b.tile([P, 4, 512], F32, tag="s")
                        for j in range(4):
                            m = 4 * g + j
                            nc.tensor.matmul(
                                S[:, j, :],
                                KT_s[base:base + DH, m * P:(m + 1) * P],
                                QT_s[base:base + DH, ch * 512:(ch + 1) * 512],
                                start=True, stop=True)
                        E8 = estream.tile([P, 4, 512], F8, tag="e")
                        nc.scalar.activation(out=E8, in_=S, func=AF.Exp, scale=SCALE)
                        for j2 in range(2):
                            nc.tensor.matmul(
                                y, Vf8[:, 4 * g + 2 * j2:4 * g + 2 * j2 + 2, h, :],
                                E8[:, 2 * j2:2 * j2 + 2, :],
                                start=(g == 0 and j2 == 0),
                                stop=(g == 3 and j2 == 1), perf_mode=DR)
                    # normalize: rinv broadcast via PE ones outer product, copy to
                    # SBUF (DVE pairs with one PSUM operand max), one multiply
                    rin = work.tile([P, 512], BF, tag="ri")
                    with nc.allow_low_precision(reason="bf16 softmax denom recip"):
                        nc.vector.reciprocal(out=rin[DH:DH + 1, :], in_=y[DH:DH + 1, :])
                    rb = ps_b.tile([DH, 512], F32, tag="rb")
                    nc.tensor.matmul(rb, ones_bf[DH:DH + 1, 0:DH],
                                     rin[DH:DH + 1, :], start=True, stop=True)
                    rbs = work.tile([DH, 512], BF, tag="rb")
                    nc.vector.tensor_copy(out=rbs, in_=rb)
                    nc.vector.tensor_tensor(
                        out=Yf8[:, h, ch * 512:(ch + 1) * 512],
                        in0=y[0:DH, :], in1=rbs, op=A.mult)

        # ---------------- Phase C: proj -> + x + b_proj -> x2
        with tc.tile_pool(name="ps_c", bufs=1, space="PSUM") as ps_c:
            for th in range(2):
                attnT = work.tile([P, CT, 512], BF, tag="at")
                for oc in range(CT):
                    pps = ps_c.tile([P, 512], F32, tag="p", bufs=2)
                    for j in range(CT):
                        nc.tensor.matmul(
                            pps, Wp_s[:, 2 * j:2 * j + 2, oc * P:(oc + 1) * P],
                            Yf8[:, 2 * j:2 * j + 2, th * 512:(th + 1) * 512],
                            start=(j == 0), stop=(j == CT - 1), perf_mode=DR)
                    nc.vector.tensor_scalar(out=attnT[:, oc, :], in0=pps,
                                            scalar1=IWS, scalar2=None, op0=A.mult)
                for i in range(4):
                    it = th * 4 + i
                    tpc = ps_c.tile([P, C], BF, tag="tr", bufs=2)
                    for t in range(CT):
                        nc.tensor.transpose(tpc[:, t * P:(t + 1) * P],
                                            attnT[:, t, i * P:(i + 1) * P], ident_bf)
                    x_t = io.tile([P, C], BF, tag="x")
                    dma(out=x_t, in_=xb[it * P:(it + 1) * P, :])
                    xpb = work.tile([P, C], BF, tag="xpb")
                    nc.vector.tensor_tensor(out=xpb, in0=x_t, in1=bproj_bc, op=A.add)
                    nc.vector.tensor_tensor(out=x2[:, it, :], in0=tpc, in1=xpb,
                                            op=A.add)

        # ---------------- Phase D: LN2 -> x2lnT
        with tc.tile_pool(name="ps_d", bufs=2, space="PSUM") as ps_d:
            for i in range(NOT_):
                z2 = work.tile([P, C], BF, tag="z")
                ln_apply(x2[:, i, :], z2, C)
                tpd = ps_d.tile([P, C], BF, tag="tr")
                for t in range(CT):
                    nc.tensor.transpose(tpd[:, t * P:(t + 1) * P],
                                        z2[:, t * P:(t + 1) * P], ident_bf)
                nc.vector.tensor_copy(
                    out=x2lnT[:, :, i * P:(i + 1) * P],
                    in_=tpd[:].rearrange("p (t n) -> p t n", t=CT))

        # ---------------- Phase E: MLP + residual -> out, per 512-token half
        with tc.tile_pool(name="ps_e", bufs=1, space="PSUM") as ps_e:
            for th in range(2):
                sl = slice(th * 512, (th + 1) * 512)
                for hg in range(HT):
                    f1 = ps_e.tile([P, 512], F32, tag="f1", bufs=2)
                    for k2 in range(3):
                        nc.tensor.matmul(
                            f1, Wfc1_s[:, 2 * k2:2 * k2 + 2, hg * P:(hg + 1) * P],
                            x2lnT[:, 2 * k2:2 * k2 + 2, sl],
                            start=(k2 == 0), stop=(k2 == 2), perf_mode=DR)
                    nc.scalar.activation(out=GA8[:, hg, :], in_=f1, func=AF.Gelu,
                                         bias=f1bT[:, hg:hg + 1], scale=IWS)
                mlpT = work.tile([P, CT, 512], BF, tag="at")
                for oc in range(CT):
                    fa = ps_e.tile([P, 512], F32, tag="fa")
                    for j in range(12):
                        nc.tensor.matmul(
                            fa, Wfc2_s[:, 2 * j:2 * j + 2, oc * P:(oc + 1) * P],
                            GA8[:, 2 * j:2 * j + 2, :],
                            start=(j == 0), stop=(j == 11), perf_mode=DR)
                    nc.vector.tensor_scalar(out=mlpT[:, oc, :], in0=fa,
                                            scalar1=IWS, scalar2=bf2T[:, oc:oc + 1],
                                            op0=A.mult, op1=A.add)
                for i in range(4):
                    it = th * 4 + i
                    tpe = ps_e.tile([P, C], BF, tag="tr", bufs=2)
                    for t in range(CT):
                        nc.tensor.transpose(tpe[:, t * P:(t + 1) * P],
                                            mlpT[:, t, i * P:(i + 1) * P], ident_bf)
                    o_sb = work.tile([P, C], F32, tag="o")
                    nc.vector.tensor_tensor(out=o_sb, in0=tpe, in1=x2[:, it, :],
                                            op=A.add)
                    dma(out=out[it * P:(it + 1) * P, :], in_=o_sb)

        estream.release()
        io.release()
        work.release()
        qkpool.release()
        big.release()
        wpool.release()
        consts.release()

    nc.compile()
    return nc


_NC_CACHE = None


def kernel(x, ln1_g, ln1_b, w_qkv, w_proj, b_proj, ln2_g, ln2_b,
           w_fc1, b_fc1, w_fc2, b_fc2):
    global _NC_CACHE
    import ml_dtypes
    from concourse.bass_utils import run_bass_kernel_spmd

    F8NP = ml_dtypes.float8_e4m3
    BFNP = ml_dtypes.bfloat16

    x = np.asarray(x, np.float32)
    ln1_g = np.asarray(ln1_g, np.float32)
    ln1_b = np.asarray(ln1_b, np.float32)
    ln2_g = np.asarray(ln2_g, np.float32)
    ln2_b = np.asarray(ln2_b, np.float32)
    w_qkv = np.asarray(w_qkv, np.float32)
    w_proj = np.asarray(w_proj, np.float32)
    w_fc1 = np.asarray(w_fc1, np.float32)
    w_fc2 = np.asarray(w_fc2, np.float32)

    # host-side folding + fp8 prescaling
    w_qkv8 = np.asarray(w_qkv * ln1_g[:, None] * WS, F8NP)
    qkv_bias = (ln1_b @ w_qkv).astype(np.float32)
    # proj weights rearranged [DH, H, C] so head pairs share partitions 0:64
    w_proj8 = np.ascontiguousarray(
        np.asarray(w_proj * WS, F8NP).reshape(H, DH, C).transpose(1, 0, 2)
    ).reshape(DH, H * C)
    w_fc18 = np.asarray(w_fc1 * ln2_g[:, None] * WS, F8NP)
    fc1_bias = (ln2_b @ w_fc1 + np.asarray(b_fc1, np.float32)).astype(np.float32)
    w_fc28 = np.asarray(w_fc2 * WS, F8NP)

    shared = {
        "w_qkv8": w_qkv8,
        "w_proj8": w_proj8,
        "w_fc18": w_fc18,
        "w_fc28": w_fc28,
        "qkv_bias": qkv_bias,
        "fc1_bias": fc1_bias,
        "b_proj_bf": np.asarray(b_proj, BFNP),
        "b_fc2": np.asarray(b_fc2, np.float32),
    }
    in_maps = []
    for c in range(8):
        b, hh = c // 2, c % 2
        xbv = np.ascontiguousarray(
            np.asarray(np.roll(x[b], -hh * NO, axis=0), BFNP))
        in_maps.append({"xb": xbv, **shared})

    if _NC_CACHE is None:
        _NC_CACHE = _build_bass()
    res = run_bass_kernel_spmd(_NC_CACHE, in_maps, core_ids=list(range(8)))

    outp = np.empty((B, N, C), np.float32)
    for c in range(8):
        b, hh = c // 2, c % 2
        outp[b, hh * NO:(hh + 1) * NO, :] = res.results[c]["out"]
    return outp


# revision 8
# speedup vs baseline: 1.9037x; 1.3066x over previous
"""Trainium2 Bass kernel for a dense transformer block (B=4, N=2048, C=768, H=12).

Sharding: 8 cores = 4 batches x 2 sequence halves (queries split; K/V duplicated
per batch pair, no collectives). Each core receives its batch rolled so its own
1024 query rows are rows 0:1023.

v2 dataflow (cost-model-driven):
- All projection/attention-value/MLP matmuls run in fp8e4m3 with DoubleRow perf
  mode (2 contraction k-tiles per instruction, 0.5 cycles/column = 4x fp32r).
  Weights are folded (LN gains), prescaled by 32 on the host, and cast to fp8;
  the 1/32 unscale + bias ride the PSUM->SBUF copies (or gelu's scale/bias).
- Scores stay bf16 (precision-sensitive); softmax exp runs on the Act engine in
  [128, 4, 512] PSUM tiles (2048-column calls) writing fp8 directly; the
  denominator rides the value matmul as a 65th ones-row of V.
- Per-query 1/den is applied via a PE ones-outer-product broadcast plus one DVE
  multiply per (head, chunk) - no transposes.
- LN uses Sqrt+DVE-reciprocal (no Ln/Exp table thrash); only 4 act-table loads.
- All DMA goes through SP HWDGE (Pool engine stays free for psum->sbuf copies).
"""

import numpy as np

B, N, C = 4, 2048, 768
H, DH = 12, 64
HID = 4 * C
SCALE = DH ** -0.5
EPS = 1e-5
WS = 32.0
IWS = 1.0 / WS

P = 128
CT = C // P          # 6
NT = N // P          # 16
NO = N // 2          # 1024 own rows
NOT_ = NO // P       # 8
HT = HID // P        # 24


def _build_bass():
    import concourse.bass as bass
    import concourse.tile as tile
    from concourse import bacc, mybir
    from concourse.masks import make_identity
    from concourse.alu_op_type import AluOpType as A

    F32 = mybir.dt.float32
    BF = mybir.dt.bfloat16
    F8 = mybir.dt.float8e4
    AF = mybir.ActivationFunctionType
    DR = mybir.MatmulPerfMode.DoubleRow

    nc = bacc.Bacc("TRN2", target_bir_lowering=False, num_swdge_queues=4)

    xb = nc.dram_tensor("xb", [N, C], BF, kind="ExternalInput")
    w_qkv8 = nc.dram_tensor("w_qkv8", [C, 3 * C], F8, kind="ExternalInput")
    w_proj8 = nc.dram_tensor("w_proj8", [DH, H * C], F8, kind="ExternalInput")
    w_fc18 = nc.dram_tensor("w_fc18", [C, HID], F8, kind="ExternalInput")
    w_fc28 = nc.dram_tensor("w_fc28", [HID, C], F8, kind="ExternalInput")
    qkv_bias = nc.dram_tensor("qkv_bias", [3 * C], F32, kind="ExternalInput")
    fc1_bias = nc.dram_tensor("fc1_bias", [HID], F32, kind="ExternalInput")
    b_proj_bf = nc.dram_tensor("b_proj_bf", [C], BF, kind="ExternalInput")
    b_fc2 = nc.dram_tensor("b_fc2", [C], F32, kind="ExternalInput")
    out = nc.dram_tensor("out", [NO, C], F32, kind="ExternalOutput")

    dma = nc.sync.dma_start

    with tile.TileContext(nc) as tc:
        consts = tc.alloc_tile_pool(name="consts", bufs=1)
        wpool = tc.alloc_tile_pool(name="wpool", bufs=1)
        big = tc.alloc_tile_pool(name="big", bufs=1)
        qkpool = tc.alloc_tile_pool(name="qkpool", bufs=2)
        work = tc.alloc_tile_pool(name="work", bufs=2)
        io = tc.alloc_tile_pool(name="io", bufs=4)
        estream = tc.alloc_tile_pool(name="estream", bufs=2)

        ident_bf = consts.tile([P, P], BF)
        make_identity(nc, ident_bf)
        ones_bf = consts.tile([P, DH], BF)
        nc.gpsimd.memset(ones_bf, 1.0)
        eps_t = consts.tile([P, 1], F32)
        nc.vector.memset(eps_t, EPS)
        qbT = consts.tile([P, 18], F32)
        dma(out=qbT, in_=qkv_bias[:].rearrange("(t p) -> p t", p=P))
        f1bT = consts.tile([P, HT], F32)
        dma(out=f1bT, in_=fc1_bias[:].rearrange("(t p) -> p t", p=P))
        bf2T = consts.tile([P, CT], F32)
        dma(out=bf2T, in_=b_fc2[:].rearrange("(t p) -> p t", p=P))
        vbias_bc = consts.tile([P, C], F32)
        dma(out=vbias_bc, in_=qkv_bias[2 * C:].partition_broadcast(P))
        bproj_bc = consts.tile([P, C], BF)
        dma(out=bproj_bc, in_=b_proj_bf[:].partition_broadcast(P))

        Wqkv_s = wpool.tile([P, CT, 3 * C], F8)
        Wp_s = wpool.tile([DH, H, C], F8)
        Wfc1_s = wpool.tile([P, CT, HID], F8)
        Wfc2_s = wpool.tile([P, HT, C], F8)

        hT = big.tile([P, CT, N], F8)          # LN1(x)^T, fp8
        VW = 80  # V cols + ones + pad: dual-fp8 ldweights needs M % 16 == 0
        Vf8 = big.tile([P, NT, H, VW], F8)
        Yf8 = big.tile([DH, H, NO], F8)         # normalized y, proj rhs
        x2lnT = big.tile([P, CT, NO], F8)       # LN2(x2)^T
        GA8 = big.tile([P, HT, 512], F8)        # gelu acts, per token-half
        x2 = big.tile([P, NOT_, C], BF)         # x + attn + b_proj

        nc.vector.memset(Vf8[:, :, :, DH:VW], 1.0)

        def ln_apply(src, dst_bf, csize):
            # dst = (src - mean) * rsqrt(var+eps); gains/biases are host-folded
            st = work.tile([P, 3, 6], F32, tag="ln_st")
            for s in range(3):
                nc.vector.bn_stats(out=st[:, s, :], in_=src[:, s * 256:(s + 1) * 256])
            mv = work.tile([P, 2], F32, tag="ln_mv")
            nc.vector.bn_aggr(out=mv, in_=st)
            sd = work.tile([P, 1], F32, tag="ln_sd")
            nc.scalar.activation(out=sd, in_=mv[:, 1:2], func=AF.Sqrt, bias=eps_t)
            r = work.tile([P, 1], F32, tag="ln_r")
            nc.vector.reciprocal(out=r, in_=sd)
            nc.vector.tensor_scalar(out=dst_bf, in0=src, scalar1=mv[:, 0:1],
                                    scalar2=r, op0=A.subtract, op1=A.mult)

        # ---------------- Phase A: LN1 -> hT (fp8, transposed)
        with tc.tile_pool(name="ps_a", bufs=3, space="PSUM") as ps_a:
            for i in range(NT):
                x_t = io.tile([P, C], BF, tag="x")
                dma(out=x_t, in_=xb[i * P:(i + 1) * P, :])
                if i == 7:
                    dma(out=Wqkv_s,
                        in_=w_qkv8[:].rearrange("(t p) j -> p t j", p=P))
                z = work.tile([P, C], BF, tag="z")
                ln_apply(x_t, z, C)
                tp = ps_a.tile([P, C], BF, tag="tr")
                for t in range(CT):
                    nc.tensor.transpose(tp[:, t * P:(t + 1) * P],
                                        z[:, t * P:(t + 1) * P], ident_bf)
                nc.vector.tensor_copy(
                    out=hT[:, :, i * P:(i + 1) * P],
                    in_=tp[:].rearrange("p (t n) -> p t n", t=CT))
        dma(out=Wp_s, in_=w_proj8[:].rearrange("d (h j) -> d h j", h=H))
        dma(out=Wfc1_s, in_=w_fc18[:].rearrange("(t p) j -> p t j", p=P))
        dma(out=Wfc2_s, in_=w_fc28[:].rearrange("(t p) j -> p t j", p=P))

        # ---------------- Phase B0: V (token-major) for all heads
        with tc.tile_pool(name="ps_v", bufs=2, space="PSUM") as ps_v:
            for i in range(NT):
                vps = ps_v.tile([P, C], F32, tag="v")
                for k2 in range(3):
                    for sl, c0 in ((512, 0), (256, 512)):
                        nc.tensor.matmul(
                            vps[:, c0:c0 + sl],
                            hT[:, 2 * k2:2 * k2 + 2, i * P:(i + 1) * P],
                            Wqkv_s[:, 2 * k2:2 * k2 + 2, 2 * C + c0:2 * C + c0 + sl],
                            start=(k2 == 0), stop=(k2 == 2), perf_mode=DR)
                nc.vector.scalar_tensor_tensor(
                    out=Vf8[:, i, :, 0:DH],
                    in0=vps[:].rearrange("p (h d) -> p h d", h=H),
                    scalar=IWS, in1=vbias_bc[:].rearrange("p (h d) -> p h d", h=H),
                    op0=A.mult, op1=A.add)

        # ---------------- Phase B: attention, head-sequential
        with tc.tile_pool(name="ps_b", bufs=1, space="PSUM") as ps_b:
            for h in range(H):
                hp, sub = h // 2, h % 2
                base = sub * DH
                if sub == 0:
                    # Q (own rows) and K (all rows) for this head pair
                    QT_s = qkpool.tile([P, NO], BF, tag="qt")
                    for q2 in range(2):
                        qps = ps_b.tile([P, 512], F32, tag="qk")
                        for k2 in range(3):
                            nc.tensor.matmul(
                                qps,
                                Wqkv_s[:, 2 * k2:2 * k2 + 2, hp * P:(hp + 1) * P],
                                hT[:, 2 * k2:2 * k2 + 2, q2 * 512:(q2 + 1) * 512],
                                start=(k2 == 0), stop=(k2 == 2), perf_mode=DR)
                        nc.vector.tensor_scalar(
                            out=QT_s[:, q2 * 512:(q2 + 1) * 512], in0=qps,
                            scalar1=IWS, scalar2=qbT[:, hp:hp + 1],
                            op0=A.mult, op1=A.add)
                    KT_s = qkpool.tile([P, N], BF, tag="kt")
                    for q4 in range(4):
                        kps = ps_b.tile([P, 512], F32, tag="qk")
                        for k2 in range(3):
                            nc.tensor.matmul(
                                kps,
                                Wqkv_s[:, 2 * k2:2 * k2 + 2, C + hp * P:C + (hp + 1) * P],
                                hT[:, 2 * k2:2 * k2 + 2, q4 * 512:(q4 + 1) * 512],
                                start=(k2 == 0), stop=(k2 == 2), perf_mode=DR)
                        nc.vector.tensor_scalar(
                            out=KT_s[:, q4 * 512:(q4 + 1) * 512], in0=kps,
                            scalar1=IWS, scalar2=qbT[:, 6 + hp:7 + hp],
                            op0=A.mult, op1=A.add)
                for ch in range(2):
                    y = ps_b.tile([80, 512], F32, tag="y", bufs=2)
                    for g in range(8):
                        S = ps_b.tile([P, 2, 512], F32, tag="s", bufs=2)
                        for j in range(2):
                            m = 2 * g + j
                            nc.tensor.matmul(
                                S[:, j, :],
                                KT_s[base:base + DH, m * P:(m + 1) * P],
                                QT_s[base:base + DH, ch * 512:(ch + 1) * 512],
                                start=True, stop=True)
                        E8 = estream.tile([P, 2, 512], F8, tag="e")
                        nc.scalar.activation(out=E8, in_=S, func=AF.Exp, scale=SCALE)
                        nc.tensor.matmul(
                            y, Vf8[:, 2 * g:2 * g + 2, h, :], E8,
                            start=(g == 0), stop=(g == 7), perf_mode=DR)
                    # normalize: rinv broadcast via PE ones outer product, copy to
                    # SBUF (DVE pairs with one PSUM operand max), one multiply
                    rin = work.tile([P, 512], BF, tag="ri")
                    with nc.allow_low_precision(reason="bf16 softmax denom recip"):
                        nc.vector.reciprocal(out=rin[DH:DH + 1, :], in_=y[DH:DH + 1, :])
                    rb = ps_b.tile([DH, 512], F32, tag="rb")
                    nc.tensor.matmul(rb, ones_bf[DH:DH + 1, 0:DH],
                                     rin[DH:DH + 1, :], start=True, stop=True)
                    rbs = work.tile([DH, 512], BF, tag="rb")
                    nc.vector.tensor_copy(out=rbs, in_=rb)
                    nc.vector.tensor_tensor(
                        out=Yf8[:, h, ch * 512:(ch + 1) * 512],
                        in0=y[0:DH, :], in1=rbs, op=A.mult)

        # ---------------- Phase C: proj -> + x + b_proj -> x2
        with tc.tile_pool(name="ps_c", bufs=1, space="PSUM") as ps_c:
            for th in range(2):
                attnT = work.tile([P, CT, 512], BF, tag="at")
                for oc in range(CT):
                    pps = ps_c.tile([P, 512], F32, tag="p", bufs=2)
                    for j in range(CT):
                        nc.tensor.matmul(
                            pps, Wp_s[:, 2 * j:2 * j + 2, oc * P:(oc + 1) * P],
                            Yf8[:, 2 * j:2 * j + 2, th * 512:(th + 1) * 512],
                            start=(j == 0), stop=(j == CT - 1), perf_mode=DR)
                    nc.vector.tensor_scalar(out=attnT[:, oc, :], in0=pps,
                                            scalar1=IWS, scalar2=None, op0=A.mult)
                for i in range(4):
                    it = th * 4 + i
                    tpc = ps_c.tile([P, C], BF, tag="tr", bufs=2)
                    for t in range(CT):
                        nc.tensor.transpose(tpc[:, t * P:(t + 1) * P],
                                            attnT[:, t, i * P:(i + 1) * P], ident_bf)
                    x_t = io.tile([P, C], BF, tag="x")
                    dma(out=x_t, in_=xb[it * P:(it + 1) * P, :])
                    xpb = work.tile([P, C], BF, tag="xpb")
                    nc.vector.tensor_tensor(out=xpb, in0=x_t, in1=bproj_bc, op=A.add)
                    nc.vector.tensor_tensor(out=x2[:, it, :], in0=tpc, in1=xpb,
                                            op=A.add)

        # ---------------- Phase D: LN2 -> x2lnT
        with tc.tile_pool(name="ps_d", bufs=2, space="PSUM") as ps_d:
            for i in range(NOT_):
                z2 = work.tile([P, C], BF, tag="z")
                ln_apply(x2[:, i, :], z2, C)
                tpd = ps_d.tile([P, C], BF, tag="tr")
                for t in range(CT):
                    nc.tensor.transpose(tpd[:, t * P:(t + 1) * P],
                                        z2[:, t * P:(t + 1) * P], ident_bf)
                nc.vector.tensor_copy(
                    out=x2lnT[:, :, i * P:(i + 1) * P],
                    in_=tpd[:].rearrange("p (t n) -> p t n", t=CT))

        # ---------------- Phase E: MLP + residual -> out, per 512-token half
        with tc.tile_pool(name="ps_e", bufs=1, space="PSUM") as ps_e:
            for th in range(2):
                sl = slice(th * 512, (th + 1) * 512)
                for hg in range(HT):
                    f1 = ps_e.tile([P, 512], F32, tag="f1", bufs=2)
                    for k2 in range(3):
                        nc.tensor.matmul(
                            f1, Wfc1_s[:, 2 * k2:2 * k2 + 2, hg * P:(hg + 1) * P],
                            x2lnT[:, 2 * k2:2 * k2 + 2, sl],
                            start=(k2 == 0), stop=(k2 == 2), perf_mode=DR)
                    nc.scalar.activation(out=GA8[:, hg, :], in_=f1, func=AF.Gelu,
                                         bias=f1bT[:, hg:hg + 1], scale=IWS)
                mlpT = work.tile([P, CT, 512], BF, tag="at")
                for oc in range(CT):
                    fa = ps_e.tile([P, 512], F32, tag="fa")
                    for j in range(12):
                        nc.tensor.matmul(
                            fa, Wfc2_s[:, 2 * j:2 * j + 2, oc * P:(oc + 1) * P],
                            GA8[:, 2 * j:2 * j + 2, :],
                            start=(j == 0), stop=(j == 11), perf_mode=DR)
                    nc.vector.tensor_scalar(out=mlpT[:, oc, :], in0=fa,
                                            scalar1=IWS, scalar2=bf2T[:, oc:oc + 1],
                                            op0=A.mult, op1=A.add)
                for i in range(4):
                    it = th * 4 + i
                    tpe = ps_e.tile([P, C], BF, tag="tr", bufs=2)
                    for t in range(CT):
                        nc.tensor.transpose(tpe[:, t * P:(t + 1) * P],
                                            mlpT[:, t, i * P:(i + 1) * P], ident_bf)
                    o_sb = work.tile([P, C], F32, tag="o")
                    nc.vector.tensor_tensor(out=o_sb, in0=tpe, in1=x2[:, it, :],
                                            op=A.add)
                    dma(out=out[it * P:(it + 1) * P, :], in_=o_sb)

        estream.release()
        io.release()
        work.release()
        qkpool.release()
        big.release()
        wpool.release()
        consts.release()

    nc.compile()
    return nc


_NC_CACHE = None


def kernel(x, ln1_g, ln1_b, w_qkv, w_proj, b_proj, ln2_g, ln2_b,
           w_fc1, b_fc1, w_fc2, b_fc2):
    global _NC_CACHE
    import ml_dtypes
    from concourse.bass_utils import run_bass_kernel_spmd

    F8NP = ml_dtypes.float8_e4m3
    BFNP = ml_dtypes.bfloat16

    x = np.asarray(x, np.float32)
    ln1_g = np.asarray(ln1_g, np.float32)
    ln1_b = np.asarray(ln1_b, np.float32)
    ln2_g = np.asarray(ln2_g, np.float32)
    ln2_b = np.asarray(ln2_b, np.float32)
    w_qkv = np.asarray(w_qkv, np.float32)
    w_proj = np.asarray(w_proj, np.float32)
    w_fc1 = np.asarray(w_fc1, np.float32)
    w_fc2 = np.asarray(w_fc2, np.float32)

    # host-side folding + fp8 prescaling
    w_qkv8 = np.asarray(w_qkv * ln1_g[:, None] * WS, F8NP)
    qkv_bias = (ln1_b @ w_qkv).astype(np.float32)
    # proj weights rearranged [DH, H, C] so head pairs share partitions 0:64
    w_proj8 = np.ascontiguousarray(
        np.asarray(w_proj * WS, F8NP).reshape(H, DH, C).transpose(1, 0, 2)
    ).reshape(DH, H * C)
    w_fc18 = np.asarray(w_fc1 * ln2_g[:, None] * WS, F8NP)
    fc1_bias = (ln2_b @ w_fc1 + np.asarray(b_fc1, np.float32)).astype(np.float32)
    w_fc28 = np.asarray(w_fc2 * WS, F8NP)

    shared = {
        "w_qkv8": w_qkv8,
        "w_proj8": w_proj8,
        "w_fc18": w_fc18,
        "w_fc28": w_fc28,
        "qkv_bias": qkv_bias,
        "fc1_bias": fc1_bias,
        "b_proj_bf": np.asarray(b_proj, BFNP),
        "b_fc2": np.asarray(b_fc2, np.float32),
    }
    in_maps = []
    for c in range(8):
        b, hh = c // 2, c % 2
        xbv = np.ascontiguousarray(
            np.asarray(np.roll(x[b], -hh * NO, axis=0), BFNP))
        in_maps.append({"xb": xbv, **shared})

    if _NC_CACHE is None:
        _NC_CACHE = _build_bass()
    res = run_bass_kernel_spmd(_NC_CACHE, in_maps, core_ids=list(range(8)))

    outp = np.empty((B, N, C), np.float32)
    for c in range(8):
        b, hh = c // 2, c % 2
        outp[b, hh * NO:(hh + 1) * NO, :] = res.results[c]["out"]
    return outp


# revision 9
# speedup vs baseline: 1.9733x; 1.0366x over previous
"""Trainium2 Bass kernel for a dense transformer block (B=4, N=2048, C=768, H=12).

Sharding: 8 cores = 4 batches x 2 sequence halves (queries split; K/V duplicated
per batch pair, no collectives). Each core receives its batch rolled so its own
1024 query rows are rows 0:1023.

v2 dataflow (cost-model-driven):
- All projection/attention-value/MLP matmuls run in fp8e4m3 with DoubleRow perf
  mode (2 contraction k-tiles per instruction, 0.5 cycles/column = 4x fp32r).
  Weights are folded (LN gains), prescaled by 32 on the host, and cast to fp8;
  the 1/32 unscale + bias ride the PSUM->SBUF copies (or gelu's scale/bias).
- Scores stay bf16 (precision-sensitive); softmax exp runs on the Act engine in
  [128, 4, 512] PSUM tiles (2048-column calls) writing fp8 directly; the
  denominator rides the value matmul as a 65th ones-row of V.
- Per-query 1/den is applied via a PE ones-outer-product broadcast plus one DVE
  multiply per (head, chunk) - no transposes.
- LN uses Sqrt+DVE-reciprocal (no Ln/Exp table thrash); only 4 act-table loads.
- All DMA goes through SP HWDGE (Pool engine stays free for psum->sbuf copies).
"""

import numpy as np

B, N, C = 4, 2048, 768
H, DH = 12, 64
HID = 4 * C
SCALE = DH ** -0.5
EPS = 1e-5
WS = 32.0
IWS = 1.0 / WS

P = 128
CT = C // P          # 6
NT = N // P          # 16
NO = N // 2          # 1024 own rows
NOT_ = NO // P       # 8
HT = HID // P        # 24


def _build_bass():
    import concourse.bass as bass
    import concourse.tile as tile
    from concourse import bacc, mybir
    from concourse.masks import make_identity
    from concourse.alu_op_type import AluOpType as A

    F32 = mybir.dt.float32
    BF = mybir.dt.bfloat16
    F8 = mybir.dt.float8e4
    AF = mybir.ActivationFunctionType
    DR = mybir.MatmulPerfMode.DoubleRow

    nc = bacc.Bacc("TRN2", target_bir_lowering=False, num_swdge_queues=4)

    xb = nc.dram_tensor("xb", [N, C], BF, kind="ExternalInput")
    w_qkv8 = nc.dram_tensor("w_qkv8", [C, 3 * C], F8, kind="ExternalInput")
    w_proj8 = nc.dram_tensor("w_proj8", [DH, H * C], F8, kind="ExternalInput")
    w_fc18 = nc.dram_tensor("w_fc18", [C, HID], F8, kind="ExternalInput")
    w_fc28 = nc.dram_tensor("w_fc28", [HID, C], F8, kind="ExternalInput")
    qkv_bias = nc.dram_tensor("qkv_bias", [3 * C], F32, kind="ExternalInput")
    fc1_bias = nc.dram_tensor("fc1_bias", [HID], F32, kind="ExternalInput")
    b_proj_eff = nc.dram_tensor("b_proj_eff", [C], F32, kind="ExternalInput")
    b_fc2 = nc.dram_tensor("b_fc2", [C], F32, kind="ExternalInput")
    out = nc.dram_tensor("out", [NO, C], F32, kind="ExternalOutput")

    dma = nc.sync.dma_start

    with tile.TileContext(nc) as tc:
        consts = tc.alloc_tile_pool(name="consts", bufs=1)
        wpool = tc.alloc_tile_pool(name="wpool", bufs=1)
        big = tc.alloc_tile_pool(name="big", bufs=1)
        qkpool = tc.alloc_tile_pool(name="qkpool", bufs=2)
        work = tc.alloc_tile_pool(name="work", bufs=2)
        io = tc.alloc_tile_pool(name="io", bufs=4)
        estream = tc.alloc_tile_pool(name="estream", bufs=2)

        ident_bf = consts.tile([P, P], BF)
        make_identity(nc, ident_bf)
        ones_bf = consts.tile([P, DH], BF)
        nc.gpsimd.memset(ones_bf, 1.0)
        eps_t = consts.tile([P, 1], F32)
        nc.vector.memset(eps_t, EPS)
        qbT = consts.tile([P, 18], F32)
        dma(out=qbT, in_=qkv_bias[:].rearrange("(t p) -> p t", p=P))
        f1bT = consts.tile([P, HT], F32)
        dma(out=f1bT, in_=fc1_bias[:].rearrange("(t p) -> p t", p=P))
        bf2T = consts.tile([P, CT], F32)
        dma(out=bf2T, in_=b_fc2[:].rearrange("(t p) -> p t", p=P))
        bpT = consts.tile([P, CT], F32)
        dma(out=bpT, in_=b_proj_eff[:].rearrange("(t p) -> p t", p=P))

        Wqkv_s = wpool.tile([P, CT, 3 * C], F8)
        Wp_s = wpool.tile([DH, H, C], F8)
        Wfc1_s = wpool.tile([P, CT, HID], F8)
        Wfc2_s = wpool.tile([P, HT, C], F8)

        hT = big.tile([P, CT, N], F8)          # LN1(x)^T, fp8
        VW = 80  # V cols + ones + pad: dual-fp8 ldweights needs M % 16 == 0
        Vf8 = big.tile([P, NT, H, VW], F8)
        Yf8 = big.tile([DH, H, NO], F8)         # normalized y, proj rhs
        x2lnT = big.tile([P, CT, NO], F8)       # LN2(x2)^T
        GA8 = big.tile([P, HT, 512], F8)        # gelu acts, per token-half
        x2 = big.tile([P, NOT_, C], BF)         # x + attn + b_proj

        nc.vector.memset(Vf8[:, :, :, DH:VW], 1.0)

        def ln_apply(src, dst_bf, csize):
            # dst = (src - mean) * rsqrt(var+eps); gains/biases are host-folded
            st = work.tile([P, 3, 6], F32, tag="ln_st")
            for s in range(3):
                nc.vector.bn_stats(out=st[:, s, :], in_=src[:, s * 256:(s + 1) * 256])
            mv = work.tile([P, 2], F32, tag="ln_mv")
            nc.vector.bn_aggr(out=mv, in_=st)
            sd = work.tile([P, 1], F32, tag="ln_sd")
            nc.scalar.activation(out=sd, in_=mv[:, 1:2], func=AF.Sqrt, bias=eps_t)
            r = work.tile([P, 1], F32, tag="ln_r")
            nc.vector.reciprocal(out=r, in_=sd)
            nc.vector.tensor_scalar(out=dst_bf, in0=src, scalar1=mv[:, 0:1],
                                    scalar2=r, op0=A.subtract, op1=A.mult)

        # ---------------- Phase A: LN1 -> hT (fp8, transposed)
        with tc.tile_pool(name="ps_a", bufs=3, space="PSUM") as ps_a:
            for i in range(NT):
                x_t = io.tile([P, C], BF, tag="x")
                dma(out=x_t, in_=xb[i * P:(i + 1) * P, :])
                if i == 7:
                    dma(out=Wqkv_s,
                        in_=w_qkv8[:].rearrange("(t p) j -> p t j", p=P))
                z = work.tile([P, C], BF, tag="z")
                ln_apply(x_t, z, C)
                tp = ps_a.tile([P, C], BF, tag="tr")
                for t in range(CT):
                    nc.tensor.transpose(tp[:, t * P:(t + 1) * P],
                                        z[:, t * P:(t + 1) * P], ident_bf)
                nc.scalar.activation(
                    out=hT[:, :, i * P:(i + 1) * P],
                    in_=tp[:].rearrange("p (t n) -> p t n", t=CT), func=AF.Copy)
        dma(out=Wp_s, in_=w_proj8[:].rearrange("d (h j) -> d h j", h=H))
        dma(out=Wfc1_s, in_=w_fc18[:].rearrange("(t p) j -> p t j", p=P))
        dma(out=Wfc2_s, in_=w_fc28[:].rearrange("(t p) j -> p t j", p=P))

        # ---------------- Phase B0: V (token-major) for all heads
        with tc.tile_pool(name="ps_v", bufs=2, space="PSUM") as ps_v:
            for i in range(NT):
                vps = ps_v.tile([P, C], F32, tag="v")
                for k2 in range(3):
                    for sl, c0 in ((512, 0), (256, 512)):
                        nc.tensor.matmul(
                            vps[:, c0:c0 + sl],
                            hT[:, 2 * k2:2 * k2 + 2, i * P:(i + 1) * P],
                            Wqkv_s[:, 2 * k2:2 * k2 + 2, 2 * C + c0:2 * C + c0 + sl],
                            start=(k2 == 0), stop=(k2 == 2), perf_mode=DR)
                nc.scalar.activation(
                    out=Vf8[:, i, :, 0:DH],
                    in_=vps[:].rearrange("p (h d) -> p h d", h=H),
                    func=AF.Copy, scale=IWS)

        # ---------------- Phase B: attention, head-sequential
        with tc.tile_pool(name="ps_b", bufs=1, space="PSUM") as ps_b:
            for h in range(H):
                hp, sub = h // 2, h % 2
                base = sub * DH
                if sub == 0:
                    # Q (own rows) and K (all rows) for this head pair
                    QT_s = qkpool.tile([P, NO], BF, tag="qt")
                    for q2 in range(2):
                        qps = ps_b.tile([P, 512], F32, tag="qk")
                        for k2 in range(3):
                            nc.tensor.matmul(
                                qps,
                                Wqkv_s[:, 2 * k2:2 * k2 + 2, hp * P:(hp + 1) * P],
                                hT[:, 2 * k2:2 * k2 + 2, q2 * 512:(q2 + 1) * 512],
                                start=(k2 == 0), stop=(k2 == 2), perf_mode=DR)
                        nc.vector.tensor_scalar(
                            out=QT_s[:, q2 * 512:(q2 + 1) * 512], in0=qps,
                            scalar1=IWS, scalar2=qbT[:, hp:hp + 1],
                            op0=A.mult, op1=A.add)
                    KT_s = qkpool.tile([P, N], BF, tag="kt")
                    for q4 in range(4):
                        kps = ps_b.tile([P, 512], F32, tag="qk")
                        for k2 in range(3):
                            nc.tensor.matmul(
                                kps,
                                Wqkv_s[:, 2 * k2:2 * k2 + 2, C + hp * P:C + (hp + 1) * P],
                                hT[:, 2 * k2:2 * k2 + 2, q4 * 512:(q4 + 1) * 512],
                                start=(k2 == 0), stop=(k2 == 2), perf_mode=DR)
                        nc.vector.tensor_scalar(
                            out=KT_s[:, q4 * 512:(q4 + 1) * 512], in0=kps,
                            scalar1=IWS, scalar2=qbT[:, 6 + hp:7 + hp],
                            op0=A.mult, op1=A.add)
                for ch in range(2):
                    y = ps_b.tile([80, 512], F32, tag="y", bufs=2)
                    for g in range(8):
                        S = ps_b.tile([P, 2, 512], F32, tag="s", bufs=2)
                        for j in range(2):
                            m = 2 * g + j
                            nc.tensor.matmul(
                                S[:, j, :],
                                KT_s[base:base + DH, m * P:(m + 1) * P],
                                QT_s[base:base + DH, ch * 512:(ch + 1) * 512],
                                start=True, stop=True)
                        E8 = estream.tile([P, 2, 512], F8, tag="e")
                        nc.scalar.activation(out=E8, in_=S, func=AF.Exp, scale=SCALE)
                        nc.tensor.matmul(
                            y, Vf8[:, 2 * g:2 * g + 2, h, :], E8,
                            start=(g == 0), stop=(g == 7), perf_mode=DR)
                    # normalize: rinv broadcast via PE ones outer product, copy to
                    # SBUF (DVE pairs with one PSUM operand max), one multiply
                    rin = work.tile([P, 512], BF, tag="ri")
                    with nc.allow_low_precision(reason="bf16 softmax denom recip"):
                        nc.vector.reciprocal(out=rin[DH:DH + 1, :], in_=y[DH:DH + 1, :])
                    rb = ps_b.tile([DH, 512], F32, tag="rb")
                    nc.tensor.matmul(rb, ones_bf[DH:DH + 1, 0:DH],
                                     rin[DH:DH + 1, :], start=True, stop=True)
                    rbs = work.tile([DH, 512], BF, tag="rb")
                    nc.vector.tensor_copy(out=rbs, in_=rb)
                    nc.vector.tensor_tensor(
                        out=Yf8[:, h, ch * 512:(ch + 1) * 512],
                        in0=y[0:DH, :], in1=rbs, op=A.mult)

        # ---------------- Phase C: proj -> + x + b_proj -> x2
        with tc.tile_pool(name="ps_c", bufs=1, space="PSUM") as ps_c:
            for th in range(2):
                attnT = work.tile([P, CT, 512], BF, tag="at")
                for oc in range(CT):
                    pps = ps_c.tile([P, 512], F32, tag="p", bufs=2)
                    for j in range(CT):
                        nc.tensor.matmul(
                            pps, Wp_s[:, 2 * j:2 * j + 2, oc * P:(oc + 1) * P],
                            Yf8[:, 2 * j:2 * j + 2, th * 512:(th + 1) * 512],
                            start=(j == 0), stop=(j == CT - 1), perf_mode=DR)
                    nc.vector.tensor_scalar(out=attnT[:, oc, :], in0=pps,
                                            scalar1=IWS, scalar2=bpT[:, oc:oc + 1],
                                            op0=A.mult, op1=A.add)
                for i in range(4):
                    it = th * 4 + i
                    tpc = ps_c.tile([P, C], BF, tag="tr", bufs=2)
                    for t in range(CT):
                        nc.tensor.transpose(tpc[:, t * P:(t + 1) * P],
                                            attnT[:, t, i * P:(i + 1) * P], ident_bf)
                    x_t = io.tile([P, C], BF, tag="x")
                    dma(out=x_t, in_=xb[it * P:(it + 1) * P, :])
                    nc.vector.tensor_tensor(out=x2[:, it, :], in0=tpc, in1=x_t,
                                            op=A.add)

        # ---------------- Phase D: LN2 -> x2lnT
        with tc.tile_pool(name="ps_d", bufs=2, space="PSUM") as ps_d:
            for i in range(NOT_):
                z2 = work.tile([P, C], BF, tag="z")
                ln_apply(x2[:, i, :], z2, C)
                tpd = ps_d.tile([P, C], BF, tag="tr")
                for t in range(CT):
                    nc.tensor.transpose(tpd[:, t * P:(t + 1) * P],
                                        z2[:, t * P:(t + 1) * P], ident_bf)
                nc.scalar.activation(
                    out=x2lnT[:, :, i * P:(i + 1) * P],
                    in_=tpd[:].rearrange("p (t n) -> p t n", t=CT), func=AF.Copy)

        # ---------------- Phase E: MLP + residual -> out, per 512-token half
        with tc.tile_pool(name="ps_e", bufs=1, space="PSUM") as ps_e:
            for th in range(2):
                sl = slice(th * 512, (th + 1) * 512)
                for hg in range(HT):
                    f1 = ps_e.tile([P, 512], F32, tag="f1", bufs=2)
                    for k2 in range(3):
                        nc.tensor.matmul(
                            f1, Wfc1_s[:, 2 * k2:2 * k2 + 2, hg * P:(hg + 1) * P],
                            x2lnT[:, 2 * k2:2 * k2 + 2, sl],
                            start=(k2 == 0), stop=(k2 == 2), perf_mode=DR)
                    nc.scalar.activation(out=GA8[:, hg, :], in_=f1, func=AF.Gelu,
                                         bias=f1bT[:, hg:hg + 1], scale=IWS)
                mlpT = work.tile([P, CT, 512], BF, tag="at")
                for oc in range(CT):
                    fa = ps_e.tile([P, 512], F32, tag="fa")
                    for j in range(12):
                        nc.tensor.matmul(
                            fa, Wfc2_s[:, 2 * j:2 * j + 2, oc * P:(oc + 1) * P],
                            GA8[:, 2 * j:2 * j + 2, :],
                            start=(j == 0), stop=(j == 11), perf_mode=DR)
                    nc.vector.tensor_scalar(out=mlpT[:, oc, :], in0=fa,
                                            scalar1=IWS, scalar2=bf2T[:, oc:oc + 1],
                                            op0=A.mult, op1=A.add)
                for i in range(4):
                    it = th * 4 + i
                    tpe = ps_e.tile([P, C], BF, tag="tr", bufs=2)
                    for t in range(CT):
                        nc.tensor.transpose(tpe[:, t * P:(t + 1) * P],
                                            mlpT[:, t, i * P:(i + 1) * P], ident_bf)
                    o_sb = work.tile([P, C], F32, tag="o")
                    nc.vector.tensor_tensor(out=o_sb, in0=tpe, in1=x2[:, it, :],
                                            op=A.add)
                    dma(out=out[it * P:(it + 1) * P, :], in_=o_sb)

        estream.release()
        io.release()
        work.release()
        qkpool.release()
        big.release()
        wpool.release()
        consts.release()

    nc.compile()
    return nc


_NC_CACHE = None


def kernel(x, ln1_g, ln1_b, w_qkv, w_proj, b_proj, ln2_g, ln2_b,
           w_fc1, b_fc1, w_fc2, b_fc2):
    global _NC_CACHE
    import ml_dtypes
    from concourse.bass_utils import run_bass_kernel_spmd

    F8NP = ml_dtypes.float8_e4m3
    BFNP = ml_dtypes.bfloat16

    x = np.asarray(x, np.float32)
    ln1_g = np.asarray(ln1_g, np.float32)
    ln1_b = np.asarray(ln1_b, np.float32)
    ln2_g = np.asarray(ln2_g, np.float32)
    ln2_b = np.asarray(ln2_b, np.float32)
    w_qkv = np.asarray(w_qkv, np.float32)
    w_proj = np.asarray(w_proj, np.float32)
    w_fc1 = np.asarray(w_fc1, np.float32)
    w_fc2 = np.asarray(w_fc2, np.float32)

    # host-side folding + fp8 prescaling
    w_qkv8 = np.asarray(w_qkv * ln1_g[:, None] * WS, F8NP)
    qkv_bias = (ln1_b @ w_qkv).astype(np.float32)
    # proj weights rearranged [DH, H, C] so head pairs share partitions 0:64
    w_proj8 = np.ascontiguousarray(
        np.asarray(w_proj * WS, F8NP).reshape(H, DH, C).transpose(1, 0, 2)
    ).reshape(DH, H * C)
    w_fc18 = np.asarray(w_fc1 * ln2_g[:, None] * WS, F8NP)
    fc1_bias = (ln2_b @ w_fc1 + np.asarray(b_fc1, np.float32)).astype(np.float32)
    w_fc28 = np.asarray(w_fc2 * WS, F8NP)

    shared = {
        "w_qkv8": w_qkv8,
        "w_proj8": w_proj8,
        "w_fc18": w_fc18,
        "w_fc28": w_fc28,
        "qkv_bias": qkv_bias,
        "fc1_bias": fc1_bias,
        "b_proj_eff": (np.asarray(b_proj, np.float32)
                       + qkv_bias[2 * C:] @ w_proj).astype(np.float32),
        "b_fc2": np.asarray(b_fc2, np.float32),
    }
    in_maps = []
    for c in range(8):
        b, hh = c // 2, c % 2
        xbv = np.ascontiguousarray(
            np.asarray(np.roll(x[b], -hh * NO, axis=0), BFNP))
        in_maps.append({"xb": xbv, **shared})

    if _NC_CACHE is None:
        _NC_CACHE = _build_bass()
    res = run_bass_kernel_spmd(_NC_CACHE, in_maps, core_ids=list(range(8)))

    outp = np.empty((B, N, C), np.float32)
    for c in range(8):
        b, hh = c // 2, c % 2
        outp[b, hh * NO:(hh + 1) * NO, :] = res.results[c]["out"]
    return outp


# revision 10
# speedup vs baseline: 1.9851x; 1.0059x over previous
"""Trainium2 Bass kernel for a dense transformer block (B=4, N=2048, C=768, H=12).

Sharding: 8 cores = 4 batches x 2 sequence halves (queries split; K/V duplicated
per batch pair, no collectives). Each core receives its batch rolled so its own
1024 query rows are rows 0:1023.

v2 dataflow (cost-model-driven):
- All projection/attention-value/MLP matmuls run in fp8e4m3 with DoubleRow perf
  mode (2 contraction k-tiles per instruction, 0.5 cycles/column = 4x fp32r).
  Weights are folded (LN gains), prescaled by 32 on the host, and cast to fp8;
  the 1/32 unscale + bias ride the PSUM->SBUF copies (or gelu's scale/bias).
- Scores stay bf16 (precision-sensitive); softmax exp runs on the Act engine in
  [128, 4, 512] PSUM tiles (2048-column calls) writing fp8 directly; the
  denominator rides the value matmul as a 65th ones-row of V.
- Per-query 1/den is applied via a PE ones-outer-product broadcast plus one DVE
  multiply per (head, chunk) - no transposes.
- LN uses Sqrt+DVE-reciprocal (no Ln/Exp table thrash); only 4 act-table loads.
- All DMA goes through SP HWDGE (Pool engine stays free for psum->sbuf copies).
"""

import numpy as np

B, N, C = 4, 2048, 768
H, DH = 12, 64
HID = 4 * C
SCALE = DH ** -0.5
EPS = 1e-5
WS = 32.0
IWS = 1.0 / WS

P = 128
CT = C // P          # 6
NT = N // P          # 16
NO = N // 2          # 1024 own rows
NOT_ = NO // P       # 8
HT = HID // P        # 24


def _build_bass():
    import concourse.bass as bass
    import concourse.tile as tile
    from concourse import bacc, mybir
    from concourse.masks import make_identity
    from concourse.alu_op_type import AluOpType as A

    F32 = mybir.dt.float32
    BF = mybir.dt.bfloat16
    F8 = mybir.dt.float8e4
    AF = mybir.ActivationFunctionType
    DR = mybir.MatmulPerfMode.DoubleRow

    nc = bacc.Bacc("TRN2", target_bir_lowering=False, num_swdge_queues=4)

    xb = nc.dram_tensor("xb", [N, C], BF, kind="ExternalInput")
    w_qkv8 = nc.dram_tensor("w_qkv8", [C, 3 * C], F8, kind="ExternalInput")
    w_proj8 = nc.dram_tensor("w_proj8", [DH, H * C], F8, kind="ExternalInput")
    w_fc18 = nc.dram_tensor("w_fc18", [C, HID], F8, kind="ExternalInput")
    w_fc28 = nc.dram_tensor("w_fc28", [HID, C], F8, kind="ExternalInput")
    qkv_bias = nc.dram_tensor("qkv_bias", [3 * C], F32, kind="ExternalInput")
    fc1_bias = nc.dram_tensor("fc1_bias", [HID], F32, kind="ExternalInput")
    b_proj_eff = nc.dram_tensor("b_proj_eff", [C], F32, kind="ExternalInput")
    b_fc2 = nc.dram_tensor("b_fc2", [C], F32, kind="ExternalInput")
    out = nc.dram_tensor("out", [NO, C], F32, kind="ExternalOutput")

    dma = nc.sync.dma_start

    with tile.TileContext(nc) as tc:
        consts = tc.alloc_tile_pool(name="consts", bufs=1)
        wpool = tc.alloc_tile_pool(name="wpool", bufs=1)
        big = tc.alloc_tile_pool(name="big", bufs=1)
        qkpool = tc.alloc_tile_pool(name="qkpool", bufs=2)
        work = tc.alloc_tile_pool(name="work", bufs=2)
        io = tc.alloc_tile_pool(name="io", bufs=4)
        estream = tc.alloc_tile_pool(name="estream", bufs=2)

        ident_bf = consts.tile([P, P], BF)
        make_identity(nc, ident_bf)
        ones_bf = consts.tile([P, DH], BF)
        nc.gpsimd.memset(ones_bf, 1.0)
        eps_t = consts.tile([P, 1], F32)
        nc.vector.memset(eps_t, EPS)
        qbT = consts.tile([P, 18], F32)
        dma(out=qbT, in_=qkv_bias[:].rearrange("(t p) -> p t", p=P))
        f1bT = consts.tile([P, HT], F32)
        dma(out=f1bT, in_=fc1_bias[:].rearrange("(t p) -> p t", p=P))
        bf2T = consts.tile([P, CT], F32)
        dma(out=bf2T, in_=b_fc2[:].rearrange("(t p) -> p t", p=P))
        bpT = consts.tile([P, CT], F32)
        dma(out=bpT, in_=b_proj_eff[:].rearrange("(t p) -> p t", p=P))

        Wqkv_s = wpool.tile([P, CT, 3 * C], F8)
        Wp_s = wpool.tile([DH, H, C], F8)
        Wfc1_s = wpool.tile([P, CT, HID], F8)
        Wfc2_s = wpool.tile([P, HT, C], F8)

        hT = big.tile([P, CT, N], F8)          # LN1(x)^T, fp8
        VW = 80  # V cols + ones + pad: dual-fp8 ldweights needs M % 16 == 0
        Vf8 = big.tile([P, NT, H, VW], F8)
        Yf8 = big.tile([DH, H, NO], F8)         # normalized y, proj rhs
        x2lnT = big.tile([P, CT, NO], F8)       # LN2(x2)^T
        x2 = big.tile([P, NOT_, C], BF)         # x + attn + b_proj

        nc.vector.memset(Vf8[:, :, :, DH:VW], 1.0)

        def ln_apply(src, dst_bf, csize):
            # dst = (src - mean) * rsqrt(var+eps); gains/biases are host-folded
            st = work.tile([P, 3, 6], F32, tag="ln_st")
            for s in range(3):
                nc.vector.bn_stats(out=st[:, s, :], in_=src[:, s * 256:(s + 1) * 256])
            mv = work.tile([P, 2], F32, tag="ln_mv")
            nc.vector.bn_aggr(out=mv, in_=st)
            sd = work.tile([P, 1], F32, tag="ln_sd")
            nc.scalar.activation(out=sd, in_=mv[:, 1:2], func=AF.Sqrt, bias=eps_t)
            r = work.tile([P, 1], F32, tag="ln_r")
            nc.vector.reciprocal(out=r, in_=sd)
            nc.gpsimd.tensor_scalar(out=dst_bf, in0=src, scalar1=mv[:, 0:1],
                                     scalar2=r, op0=A.subtract, op1=A.mult)

        # ---------------- Phase A: LN1 -> hT (fp8, transposed)
        with tc.tile_pool(name="ps_a", bufs=3, space="PSUM") as ps_a:
            for i in range(NT):
                x_t = io.tile([P, C], BF, tag="x")
                dma(out=x_t, in_=xb[i * P:(i + 1) * P, :])
                if i == 7:
                    dma(out=Wqkv_s,
                        in_=w_qkv8[:].rearrange("(t p) j -> p t j", p=P))
                z = work.tile([P, C], BF, tag="z")
                ln_apply(x_t, z, C)
                tp = ps_a.tile([P, C], BF, tag="tr")
                for t in range(CT):
                    nc.tensor.transpose(tp[:, t * P:(t + 1) * P],
                                        z[:, t * P:(t + 1) * P], ident_bf)
                nc.scalar.activation(
                    out=hT[:, :, i * P:(i + 1) * P],
                    in_=tp[:].rearrange("p (t n) -> p t n", t=CT), func=AF.Copy)
        dma(out=Wp_s, in_=w_proj8[:].rearrange("d (h j) -> d h j", h=H))
        dma(out=Wfc1_s, in_=w_fc18[:].rearrange("(t p) j -> p t j", p=P))
        dma(out=Wfc2_s, in_=w_fc28[:].rearrange("(t p) j -> p t j", p=P))

        qk_tiles = {}

        def qk_produce(hp, pool):
            # Q (own rows) and K (all rows) for head pair hp
            QT_s = qkpool.tile([P, NO], BF, tag="qt")
            for q2 in range(2):
                qps = pool.tile([P, 512], F32, tag="qk")
                for k2 in range(3):
                    nc.tensor.matmul(
                        qps,
                        Wqkv_s[:, 2 * k2:2 * k2 + 2, hp * P:(hp + 1) * P],
                        hT[:, 2 * k2:2 * k2 + 2, q2 * 512:(q2 + 1) * 512],
                        start=(k2 == 0), stop=(k2 == 2), perf_mode=DR)
                nc.vector.tensor_scalar(
                    out=QT_s[:, q2 * 512:(q2 + 1) * 512], in0=qps,
                    scalar1=IWS, scalar2=qbT[:, hp:hp + 1],
                    op0=A.mult, op1=A.add)
            KT_s = qkpool.tile([P, N], BF, tag="kt")
            for q4 in range(4):
                kps = pool.tile([P, 512], F32, tag="qk")
                for k2 in range(3):
                    nc.tensor.matmul(
                        kps,
                        Wqkv_s[:, 2 * k2:2 * k2 + 2, C + hp * P:C + (hp + 1) * P],
                        hT[:, 2 * k2:2 * k2 + 2, q4 * 512:(q4 + 1) * 512],
                        start=(k2 == 0), stop=(k2 == 2), perf_mode=DR)
                nc.vector.tensor_scalar(
                    out=KT_s[:, q4 * 512:(q4 + 1) * 512], in0=kps,
                    scalar1=IWS, scalar2=qbT[:, 6 + hp:7 + hp],
                    op0=A.mult, op1=A.add)
            qk_tiles[hp] = (QT_s, KT_s)

        # ---------------- Phase B0: Q/K for pair 0, then V for all heads
        with tc.tile_pool(name="ps_v", bufs=2, space="PSUM") as ps_v:
            qk_produce(0, ps_v)
            for i in range(NT):
                vps = ps_v.tile([P, C], F32, tag="v")
                for k2 in range(3):
                    for sl, c0 in ((512, 0), (256, 512)):
                        nc.tensor.matmul(
                            vps[:, c0:c0 + sl],
                            hT[:, 2 * k2:2 * k2 + 2, i * P:(i + 1) * P],
                            Wqkv_s[:, 2 * k2:2 * k2 + 2, 2 * C + c0:2 * C + c0 + sl],
                            start=(k2 == 0), stop=(k2 == 2), perf_mode=DR)
                nc.vector.tensor_scalar(
                    out=Vf8[:, i, :, 0:DH],
                    in0=vps[:].rearrange("p (h d) -> p h d", h=H),
                    scalar1=IWS, scalar2=None, op0=A.mult)

        # ---------------- Phase B: attention, head-sequential
        with tc.tile_pool(name="ps_b", bufs=1, space="PSUM") as ps_b:
            for h in range(H):
                hp, sub = h // 2, h % 2
                base = sub * DH
                if sub == 0 and hp > 0:
                    qk_produce(hp, ps_b)
                QT_s, KT_s = qk_tiles[hp]
                for ch in range(2):
                    y = ps_b.tile([80, 512], F32, tag="y", bufs=2)
                    for g in range(8):
                        S = ps_b.tile([P, 2, 512], F32, tag="s", bufs=2)
                        for j in range(2):
                            m = 2 * g + j
                            nc.tensor.matmul(
                                S[:, j, :],
                                KT_s[base:base + DH, m * P:(m + 1) * P],
                                QT_s[base:base + DH, ch * 512:(ch + 1) * 512],
                                start=True, stop=True)
                        E8 = estream.tile([P, 2, 512], F8, tag="e")
                        nc.scalar.activation(out=E8, in_=S, func=AF.Exp, scale=SCALE)
                        nc.tensor.matmul(
                            y, Vf8[:, 2 * g:2 * g + 2, h, :], E8,
                            start=(g == 0), stop=(g == 7), perf_mode=DR)
                    # normalize: rinv broadcast via PE ones outer product, copy to
                    # SBUF (DVE pairs with one PSUM operand max), one multiply
                    rin = work.tile([P, 512], BF, tag="ri")
                    with nc.allow_low_precision(reason="bf16 softmax denom recip"):
                        nc.vector.reciprocal(out=rin[DH:DH + 1, :], in_=y[DH:DH + 1, :])
                    rb = ps_b.tile([DH, 512], F32, tag="rb")
                    nc.tensor.matmul(rb, ones_bf[DH:DH + 1, 0:DH],
                                     rin[DH:DH + 1, :], start=True, stop=True)
                    rbs = work.tile([DH, 512], BF, tag="rb")
                    nc.vector.tensor_copy(out=rbs, in_=rb)
                    nc.vector.tensor_tensor(
                        out=Yf8[:, h, ch * 512:(ch + 1) * 512],
                        in0=y[0:DH, :], in1=rbs, op=A.mult)

        # ---------------- Phase C: proj -> + x + b_proj -> x2
        with tc.tile_pool(name="ps_c", bufs=1, space="PSUM") as ps_c:
            for th in range(2):
                attnT = work.tile([P, CT, 512], BF, tag="at")
                for oc in range(CT):
                    pps = ps_c.tile([P, 512], F32, tag="p", bufs=2)
                    for j in range(CT):
                        nc.tensor.matmul(
                            pps, Wp_s[:, 2 * j:2 * j + 2, oc * P:(oc + 1) * P],
                            Yf8[:, 2 * j:2 * j + 2, th * 512:(th + 1) * 512],
                            start=(j == 0), stop=(j == CT - 1), perf_mode=DR)
                    nc.vector.tensor_scalar(out=attnT[:, oc, :], in0=pps,
                                            scalar1=IWS, scalar2=bpT[:, oc:oc + 1],
                                            op0=A.mult, op1=A.add)
                for i in range(4):
                    it = th * 4 + i
                    tpc = ps_c.tile([P, C], BF, tag="tr", bufs=2)
                    for t in range(CT):
                        nc.tensor.transpose(tpc[:, t * P:(t + 1) * P],
                                            attnT[:, t, i * P:(i + 1) * P], ident_bf)
                    x_t = io.tile([P, C], BF, tag="x")
                    dma(out=x_t, in_=xb[it * P:(it + 1) * P, :])
                    nc.vector.tensor_tensor(out=x2[:, it, :], in0=tpc, in1=x_t,
                                            op=A.add)

        # ---------------- Phase D: LN2 -> x2lnT
        with tc.tile_pool(name="ps_d", bufs=2, space="PSUM") as ps_d:
            for i in range(NOT_):
                z2 = work.tile([P, C], BF, tag="z")
                ln_apply(x2[:, i, :], z2, C)
                tpd = ps_d.tile([P, C], BF, tag="tr")
                for t in range(CT):
                    nc.tensor.transpose(tpd[:, t * P:(t + 1) * P],
                                        z2[:, t * P:(t + 1) * P], ident_bf)
                nc.scalar.activation(
                    out=x2lnT[:, :, i * P:(i + 1) * P],
                    in_=tpd[:].rearrange("p (t n) -> p t n", t=CT), func=AF.Copy)

        # ---------------- Phase E: MLP + residual -> out, per 512-token half
        with tc.tile_pool(name="ps_e", bufs=1, space="PSUM") as ps_e:
            for th in range(2):
                sl = slice(th * 512, (th + 1) * 512)
                GA8 = big.tile([P, HT, 512], F8, tag="ga", bufs=2)
                for hg in range(HT):
                    f1 = ps_e.tile([P, 512], F32, tag="f1", bufs=2)
                    for k2 in range(3):
                        nc.tensor.matmul(
                            f1, Wfc1_s[:, 2 * k2:2 * k2 + 2, hg * P:(hg + 1) * P],
                            x2lnT[:, 2 * k2:2 * k2 + 2, sl],
                            start=(k2 == 0), stop=(k2 == 2), perf_mode=DR)
                    nc.scalar.activation(out=GA8[:, hg, :], in_=f1, func=AF.Gelu,
                                         bias=f1bT[:, hg:hg + 1], scale=IWS)
                mlpT = work.tile([P, CT, 512], BF, tag="at")
                for oc in range(CT):
                    fa = ps_e.tile([P, 512], F32, tag="fa")
                    for j in range(12):
                        nc.tensor.matmul(
                            fa, Wfc2_s[:, 2 * j:2 * j + 2, oc * P:(oc + 1) * P],
                            GA8[:, 2 * j:2 * j + 2, :],
                            start=(j == 0), stop=(j == 11), perf_mode=DR)
                    nc.vector.tensor_scalar(out=mlpT[:, oc, :], in0=fa,
                                            scalar1=IWS, scalar2=bf2T[:, oc:oc + 1],
                                            op0=A.mult, op1=A.add)
                for i in range(4):
                    it = th * 4 + i
                    tpe = ps_e.tile([P, C], BF, tag="tr", bufs=2)
                    for t in range(CT):
                        nc.tensor.transpose(tpe[:, t * P:(t + 1) * P],
                                            mlpT[:, t, i * P:(i + 1) * P], ident_bf)
                    o_sb = work.tile([P, C], F32, tag="o")
                    nc.vector.tensor_tensor(out=o_sb, in0=tpe, in1=x2[:, it, :],
                                            op=A.add)
                    dma(out=out[it * P:(it + 1) * P, :], in_=o_sb)

        estream.release()
        io.release()
        work.release()
        qkpool.release()
        big.release()
        wpool.release()
        consts.release()

    nc.compile()
    return nc


_NC_CACHE = None


def kernel(x, ln1_g, ln1_b, w_qkv, w_proj, b_proj, ln2_g, ln2_b,
           w_fc1, b_fc1, w_fc2, b_fc2):
    global _NC_CACHE
    import ml_dtypes
    from concourse.bass_utils import run_bass_kernel_spmd

    F8NP = ml_dtypes.float8_e4m3
    BFNP = ml_dtypes.bfloat16

    x = np.asarray(x, np.float32)
    ln1_g = np.asarray(ln1_g, np.float32)
    ln1_b = np.asarray(ln1_b, np.float32)
    ln2_g = np.asarray(ln2_g, np.float32)
    ln2_b = np.asarray(ln2_b, np.float32)
    w_qkv = np.asarray(w_qkv, np.float32)
    w_proj = np.asarray(w_proj, np.float32)
    w_fc1 = np.asarray(w_fc1, np.float32)
    w_fc2 = np.asarray(w_fc2, np.float32)

    # host-side folding + fp8 prescaling
    w_qkv8 = np.asarray(w_qkv * ln1_g[:, None] * WS, F8NP)
    qkv_bias = (ln1_b @ w_qkv).astype(np.float32)
    # proj weights rearranged [DH, H, C] so head pairs share partitions 0:64
    w_proj8 = np.ascontiguousarray(
        np.asarray(w_proj * WS, F8NP).reshape(H, DH, C).transpose(1, 0, 2)
    ).reshape(DH, H * C)
    w_fc18 = np.asarray(w_fc1 * ln2_g[:, None] * WS, F8NP)
    fc1_bias = (ln2_b @ w_fc1 + np.asarray(b_fc1, np.float32)).astype(np.float32)
    w_fc28 = np.asarray(w_fc2 * WS, F8NP)

    shared = {
        "w_qkv8": w_qkv8,
        "w_proj8": w_proj8,
        "w_fc18": w_fc18,
        "w_fc28": w_fc28,
        "qkv_bias": qkv_bias,
        "fc1_bias": fc1_bias,
        "b_proj_eff": (np.asarray(b_proj, np.float32)
                       + qkv_bias[2 * C:] @ w_proj).astype(np.float32),
        "b_fc2": np.asarray(b_fc2, np.float32),
    }
    in_maps = []
    for c in range(8):
        b, hh = c // 2, c % 2
        xbv = np.ascontiguousarray(
            np.asarray(np.roll(x[b], -hh * NO, axis=0), BFNP))
        in_maps.append({"xb": xbv, **shared})

    if _NC_CACHE is None:
        _NC_CACHE = _build_bass()
    res = run_bass_kernel_spmd(_NC_CACHE, in_maps, core_ids=list(range(8)))

    outp = np.empty((B, N, C), np.float32)
    for c in range(8):
        b, hh = c // 2, c % 2
        outp[b, hh * NO:(hh + 1) * NO, :] = res.results[c]["out"]
    return outp


# revision 13
# speedup vs baseline: 2.0315x; 1.0234x over previous
"""Trainium2 Bass kernel for a dense transformer block (B=4, N=2048, C=768, H=12).

Sharding: 8 cores = 4 batches x 2 sequence halves (queries split; K/V duplicated
per batch pair, no collectives). Each core receives its batch rolled so its own
1024 query rows are rows 0:1023.

v2 dataflow (cost-model-driven):
- All projection/attention-value/MLP matmuls run in fp8e4m3 with DoubleRow perf
  mode (2 contraction k-tiles per instruction, 0.5 cycles/column = 4x fp32r).
  Weights are folded (LN gains), prescaled by 32 on the host, and cast to fp8;
  the 1/32 unscale + bias ride the PSUM->SBUF copies (or gelu's scale/bias).
- Scores stay bf16 (precision-sensitive); softmax exp runs on the Act engine in
  [128, 4, 512] PSUM tiles (2048-column calls) writing fp8 directly; the
  denominator rides the value matmul as a 65th ones-row of V.
- Per-query 1/den is applied via a PE ones-outer-product broadcast plus one DVE
  multiply per (head, chunk) - no transposes.
- LN uses Sqrt+DVE-reciprocal (no Ln/Exp table thrash); only 4 act-table loads.
- All DMA goes through SP HWDGE (Pool engine stays free for psum->sbuf copies).
"""

import numpy as np

B, N, C = 4, 2048, 768
H, DH = 12, 64
HID = 4 * C
SCALE = DH ** -0.5
EPS = 1e-5
WS = 32.0
IWS = 1.0 / WS

P = 128
CT = C // P          # 6
NT = N // P          # 16
NO = N // 2          # 1024 own rows
NOT_ = NO // P       # 8
HT = HID // P        # 24


def _build_bass():
    import concourse.bass as bass
    import concourse.tile as tile
    from concourse import bacc, mybir
    from concourse.masks import make_identity
    from concourse.alu_op_type import AluOpType as A

    F32 = mybir.dt.float32
    BF = mybir.dt.bfloat16
    F8 = mybir.dt.float8e4
    AF = mybir.ActivationFunctionType
    DR = mybir.MatmulPerfMode.DoubleRow

    nc = bacc.Bacc("TRN2", target_bir_lowering=False, num_swdge_queues=4)

    xb = nc.dram_tensor("xb", [N, C], BF, kind="ExternalInput")
    w_qkv8 = nc.dram_tensor("w_qkv8", [C, 3 * C], F8, kind="ExternalInput")
    w_proj8 = nc.dram_tensor("w_proj8", [DH, H * C], F8, kind="ExternalInput")
    w_fc18 = nc.dram_tensor("w_fc18", [C, HID], F8, kind="ExternalInput")
    w_fc28 = nc.dram_tensor("w_fc28", [HID, C], F8, kind="ExternalInput")
    qkv_bias = nc.dram_tensor("qkv_bias", [3 * C], F32, kind="ExternalInput")
    fc1_bias = nc.dram_tensor("fc1_bias", [HID], F32, kind="ExternalInput")
    b_proj_eff = nc.dram_tensor("b_proj_eff", [C], F32, kind="ExternalInput")
    b_fc2 = nc.dram_tensor("b_fc2", [C], F32, kind="ExternalInput")
    out = nc.dram_tensor("out", [NO, C], F32, kind="ExternalOutput")

    dma = nc.sync.dma_start

    with tile.TileContext(nc) as tc:
        consts = tc.alloc_tile_pool(name="consts", bufs=1)
        wpool = tc.alloc_tile_pool(name="wpool", bufs=1)
        big = tc.alloc_tile_pool(name="big", bufs=1)
        qkpool = tc.alloc_tile_pool(name="qkpool", bufs=2)
        work = tc.alloc_tile_pool(name="work", bufs=2)
        io = tc.alloc_tile_pool(name="io", bufs=4)
        estream = tc.alloc_tile_pool(name="estream", bufs=2)

        ident_bf = consts.tile([P, P], BF)
        make_identity(nc, ident_bf)
        ones_bf = consts.tile([P, DH], BF)
        nc.gpsimd.memset(ones_bf, 1.0)
        eps_t = consts.tile([P, 1], F32)
        nc.vector.memset(eps_t, EPS)
        qbT = consts.tile([P, 18], F32)
        dma(out=qbT, in_=qkv_bias[:].rearrange("(t p) -> p t", p=P))
        f1bT = consts.tile([P, HT], F32)
        dma(out=f1bT, in_=fc1_bias[:].rearrange("(t p) -> p t", p=P))
        bf2T = consts.tile([P, CT], F32)
        dma(out=bf2T, in_=b_fc2[:].rearrange("(t p) -> p t", p=P))
        bpT = consts.tile([P, CT], F32)
        dma(out=bpT, in_=b_proj_eff[:].rearrange("(t p) -> p t", p=P))

        Wqkv_s = wpool.tile([P, CT, 3 * C], F8)
        Wp_s = wpool.tile([DH, H, C], F8)
        Wfc1_s = wpool.tile([P, CT, HID], F8)
        Wfc2_s = wpool.tile([P, HT, C], F8)

        hT = big.tile([P, CT, N], F8)          # LN1(x)^T, fp8
        VW = 80  # V cols + ones + pad: dual-fp8 ldweights needs M % 16 == 0
        Vf8 = big.tile([P, NT, H, VW], F8)
        Yf8 = big.tile([DH, H, NO], F8)         # normalized y, proj rhs
        x2lnT = big.tile([P, CT, NO], F8)       # LN2(x2)^T
        x2 = big.tile([P, NOT_, C], BF)         # x + attn + b_proj

        nc.vector.memset(Vf8[:, :, :, DH:VW], 1.0)

        def ln_apply(src, dst_bf, csize):
            # dst = (src - mean) * rsqrt(var+eps); gains/biases are host-folded
            st = work.tile([P, 3, 6], F32, tag="ln_st")
            for s in range(3):
                nc.vector.bn_stats(out=st[:, s, :], in_=src[:, s * 256:(s + 1) * 256])
            mv = work.tile([P, 2], F32, tag="ln_mv")
            nc.vector.bn_aggr(out=mv, in_=st)
            sd = work.tile([P, 1], F32, tag="ln_sd")
            nc.scalar.activation(out=sd, in_=mv[:, 1:2], func=AF.Sqrt, bias=eps_t)
            r = work.tile([P, 1], F32, tag="ln_r")
            nc.vector.reciprocal(out=r, in_=sd)
            nc.gpsimd.tensor_scalar(out=dst_bf, in0=src, scalar1=mv[:, 0:1],
                                     scalar2=r, op0=A.subtract, op1=A.mult)

        # ---------------- Phase A: LN1 -> hT (fp8, transposed), V fused per tile
        # V weight columns load first (small); Q/K columns mid-loop.
        dma(out=Wqkv_s[:, :, 2 * C:],
            in_=w_qkv8[:, 2 * C:].rearrange("(t p) j -> p t j", p=P))
        with tc.tile_pool(name="ps_a", bufs=1, space="PSUM") as ps_a:
            for i in range(NT):
                x_t = io.tile([P, C], BF, tag="x")
                dma(out=x_t, in_=xb[i * P:(i + 1) * P, :])
                if i == 3:
                    dma(out=Wqkv_s[:, :, :2 * C],
                        in_=w_qkv8[:, :2 * C].rearrange("(t p) j -> p t j", p=P))
                z = work.tile([P, C], BF, tag="z")
                ln_apply(x_t, z, C)
                tp = ps_a.tile([P, C], BF, tag="tr", bufs=3)
                for t in range(CT):
                    nc.tensor.transpose(tp[:, t * P:(t + 1) * P],
                                        z[:, t * P:(t + 1) * P], ident_bf)
                nc.scalar.activation(
                    out=hT[:, :, i * P:(i + 1) * P],
                    in_=tp[:].rearrange("p (t n) -> p t n", t=CT), func=AF.Copy)
                # V for this key tile rides right behind its hT slice
                vps = ps_a.tile([P, C], F32, tag="v", bufs=2)
                for k2 in range(3):
                    for sl, c0 in ((512, 0), (256, 512)):
                        nc.tensor.matmul(
                            vps[:, c0:c0 + sl],
                            hT[:, 2 * k2:2 * k2 + 2, i * P:(i + 1) * P],
                            Wqkv_s[:, 2 * k2:2 * k2 + 2, 2 * C + c0:2 * C + c0 + sl],
                            start=(k2 == 0), stop=(k2 == 2), perf_mode=DR)
                if i % 2 == 0:
                    nc.vector.tensor_scalar(
                        out=Vf8[:, i, :, 0:DH],
                        in0=vps[:].rearrange("p (h d) -> p h d", h=H),
                        scalar1=IWS, scalar2=None, op0=A.mult)
                else:
                    nc.scalar.activation(
                        out=Vf8[:, i, :, 0:DH],
                        in_=vps[:].rearrange("p (h d) -> p h d", h=H),
                        func=AF.Copy, scale=IWS)
        dma(out=Wp_s, in_=w_proj8[:].rearrange("d (h j) -> d h j", h=H))
        dma(out=Wfc1_s, in_=w_fc18[:].rearrange("(t p) j -> p t j", p=P))
        dma(out=Wfc2_s, in_=w_fc28[:].rearrange("(t p) j -> p t j", p=P))

        qk_tiles = {}

        def qk_produce(hp, pool):
            # Q (own rows) and K (all rows) for head pair hp
            QT_s = qkpool.tile([P, NO], BF, tag="qt")
            for q2 in range(2):
                qps = pool.tile([P, 512], F32, tag="qk")
                for k2 in range(3):
                    nc.tensor.matmul(
                        qps,
                        Wqkv_s[:, 2 * k2:2 * k2 + 2, hp * P:(hp + 1) * P],
                        hT[:, 2 * k2:2 * k2 + 2, q2 * 512:(q2 + 1) * 512],
                        start=(k2 == 0), stop=(k2 == 2), perf_mode=DR)
                nc.vector.tensor_scalar(
                    out=QT_s[:, q2 * 512:(q2 + 1) * 512], in0=qps,
                    scalar1=IWS, scalar2=qbT[:, hp:hp + 1],
                    op0=A.mult, op1=A.add)
            KT_s = qkpool.tile([P, N], BF, tag="kt")
            for q4 in range(4):
                kps = pool.tile([P, 512], F32, tag="qk")
                for k2 in range(3):
                    nc.tensor.matmul(
                        kps,
                        Wqkv_s[:, 2 * k2:2 * k2 + 2, C + hp * P:C + (hp + 1) * P],
                        hT[:, 2 * k2:2 * k2 + 2, q4 * 512:(q4 + 1) * 512],
                        start=(k2 == 0), stop=(k2 == 2), perf_mode=DR)
                nc.vector.tensor_scalar(
                    out=KT_s[:, q4 * 512:(q4 + 1) * 512], in0=kps,
                    scalar1=IWS, scalar2=qbT[:, 6 + hp:7 + hp],
                    op0=A.mult, op1=A.add)
            qk_tiles[hp] = (QT_s, KT_s)

        # ---------------- Phase B: attention, head-sequential
        with tc.tile_pool(name="ps_b", bufs=1, space="PSUM") as ps_b:
            qk_produce(0, ps_b)
            for h in range(H):
                hp, sub = h // 2, h % 2
                base = sub * DH
                if sub == 0 and hp > 0:
                    qk_produce(hp, ps_b)
                QT_s, KT_s = qk_tiles[hp]
                for ch in range(2):
                    y = ps_b.tile([80, 512], F32, tag="y", bufs=2)
                    for g in range(8):
                        S = ps_b.tile([P, 2, 512], F32, tag="s", bufs=2)
                        for j in range(2):
                            m = 2 * g + j
                            nc.tensor.matmul(
                                S[:, j, :],
                                KT_s[base:base + DH, m * P:(m + 1) * P],
                                QT_s[base:base + DH, ch * 512:(ch + 1) * 512],
                                start=True, stop=True)
                        E8 = estream.tile([P, 2, 512], F8, tag="e")
                        nc.scalar.activation(out=E8, in_=S, func=AF.Exp, scale=SCALE)
                        nc.tensor.matmul(
                            y, Vf8[:, 2 * g:2 * g + 2, h, :], E8,
                            start=(g == 0), stop=(g == 7), perf_mode=DR)
                    # normalize: rinv broadcast via PE ones outer product, copy to
                    # SBUF (DVE pairs with one PSUM operand max), one multiply
                    rin = work.tile([P, 512], BF, tag="ri")
                    with nc.allow_low_precision(reason="bf16 softmax denom recip"):
                        nc.vector.reciprocal(out=rin[DH:DH + 1, :], in_=y[DH:DH + 1, :])
                    rb = ps_b.tile([DH, 512], F32, tag="rb")
                    nc.tensor.matmul(rb, ones_bf[DH:DH + 1, 0:DH],
                                     rin[DH:DH + 1, :], start=True, stop=True)
                    rbs = work.tile([DH, 512], BF, tag="rb")
                    nc.vector.tensor_copy(out=rbs, in_=rb)
                    nc.vector.tensor_tensor(
                        out=Yf8[:, h, ch * 512:(ch + 1) * 512],
                        in0=y[0:DH, :], in1=rbs, op=A.mult)

        # ---------------- Phase C: proj -> + x + b_proj -> x2
        with tc.tile_pool(name="ps_c", bufs=1, space="PSUM") as ps_c:
            for th in range(2):
                attnT = work.tile([P, CT, 512], BF, tag="at")
                for oc in range(CT):
                    pps = ps_c.tile([P, 512], F32, tag="p", bufs=2)
                    for j in range(CT):
                        nc.tensor.matmul(
                            pps, Wp_s[:, 2 * j:2 * j + 2, oc * P:(oc + 1) * P],
                            Yf8[:, 2 * j:2 * j + 2, th * 512:(th + 1) * 512],
                            start=(j == 0), stop=(j == CT - 1), perf_mode=DR)
                    nc.vector.tensor_scalar(out=attnT[:, oc, :], in0=pps,
                                            scalar1=IWS, scalar2=bpT[:, oc:oc + 1],
                                            op0=A.mult, op1=A.add)
                for i in range(4):
                    it = th * 4 + i
                    tpc = ps_c.tile([P, C], BF, tag="tr", bufs=2)
                    for t in range(CT):
                        nc.tensor.transpose(tpc[:, t * P:(t + 1) * P],
                                            attnT[:, t, i * P:(i + 1) * P], ident_bf)
                    x_t = io.tile([P, C], BF, tag="x")
                    dma(out=x_t, in_=xb[it * P:(it + 1) * P, :])
                    nc.vector.tensor_tensor(out=x2[:, it, :], in0=tpc, in1=x_t,
                                            op=A.add)

        # ---------------- Phase D: LN2 -> x2lnT
        with tc.tile_pool(name="ps_d", bufs=2, space="PSUM") as ps_d:
            for i in range(NOT_):
                z2 = work.tile([P, C], BF, tag="z")
                ln_apply(x2[:, i, :], z2, C)
                tpd = ps_d.tile([P, C], BF, tag="tr")
                for t in range(CT):
                    nc.tensor.transpose(tpd[:, t * P:(t + 1) * P],
                                        z2[:, t * P:(t + 1) * P], ident_bf)
                nc.scalar.activation(
                    out=x2lnT[:, :, i * P:(i + 1) * P],
                    in_=tpd[:].rearrange("p (t n) -> p t n", t=CT), func=AF.Copy)

        # ---------------- Phase E: MLP + residual -> out, per 512-token half
        with tc.tile_pool(name="ps_e", bufs=1, space="PSUM") as ps_e:
            for th in range(2):
                sl = slice(th * 512, (th + 1) * 512)
                GA8 = big.tile([P, HT, 512], F8, tag="ga", bufs=2)
                for hg in range(HT):
                    f1 = ps_e.tile([P, 512], F32, tag="f1", bufs=2)
                    for k2 in range(3):
                        nc.tensor.matmul(
                            f1, Wfc1_s[:, 2 * k2:2 * k2 + 2, hg * P:(hg + 1) * P],
                            x2lnT[:, 2 * k2:2 * k2 + 2, sl],
                            start=(k2 == 0), stop=(k2 == 2), perf_mode=DR)
                    nc.scalar.activation(out=GA8[:, hg, :], in_=f1, func=AF.Gelu,
                                         bias=f1bT[:, hg:hg + 1], scale=IWS)
                mlpT = work.tile([P, CT, 512], BF, tag="at")
                for oc in range(CT):
                    fa = ps_e.tile([P, 512], F32, tag="fa")
                    for j in range(12):
                        nc.tensor.matmul(
                            fa, Wfc2_s[:, 2 * j:2 * j + 2, oc * P:(oc + 1) * P],
                            GA8[:, 2 * j:2 * j + 2, :],
                            start=(j == 0), stop=(j == 11), perf_mode=DR)
                    nc.vector.tensor_scalar(out=mlpT[:, oc, :], in0=fa,
                                            scalar1=IWS, scalar2=bf2T[:, oc:oc + 1],
                                            op0=A.mult, op1=A.add)
                for i in range(4):
                    it = th * 4 + i
                    tpe = ps_e.tile([P, C], BF, tag="tr", bufs=2)
                    for t in range(CT):
                        nc.tensor.transpose(tpe[:, t * P:(t + 1) * P],
                                            mlpT[:, t, i * P:(i + 1) * P], ident_bf)
                    o_sb = work.tile([P, C], F32, tag="o")
                    nc.vector.tensor_tensor(out=o_sb, in0=tpe, in1=x2[:, it, :],
                                            op=A.add)
                    dma(out=out[it * P:(it + 1) * P, :], in_=o_sb)

        estream.release()
        io.release()
        work.release()
        qkpool.release()
        big.release()
        wpool.release()
        consts.release()

    nc.compile()
    return nc


_NC_CACHE = None


def kernel(x, ln1_g, ln1_b, w_qkv, w_proj, b_proj, ln2_g, ln2_b,
           w_fc1, b_fc1, w_fc2, b_fc2):
    global _NC_CACHE
    import ml_dtypes
    from concourse.bass_utils import run_bass_kernel_spmd

    F8NP = ml_dtypes.float8_e4m3
    BFNP = ml_dtypes.bfloat16

    x = np.asarray(x, np.float32)
    ln1_g = np.asarray(ln1_g, np.float32)
    ln1_b = np.asarray(ln1_b, np.float32)
    ln2_g = np.asarray(ln2_g, np.float32)
    ln2_b = np.asarray(ln2_b, np.float32)
    w_qkv = np.asarray(w_qkv, np.float32)
    w_proj = np.asarray(w_proj, np.float32)
    w_fc1 = np.asarray(w_fc1, np.float32)
    w_fc2 = np.asarray(w_fc2, np.float32)

    # host-side folding + fp8 prescaling
    w_qkv8 = np.asarray(w_qkv * ln1_g[:, None] * WS, F8NP)
    qkv_bias = (ln1_b @ w_qkv).astype(np.float32)
    # proj weights rearranged [DH, H, C] so head pairs share partitions 0:64
    w_proj8 = np.ascontiguousarray(
        np.asarray(w_proj * WS, F8NP).reshape(H, DH, C).transpose(1, 0, 2)
    ).reshape(DH, H * C)
    w_fc18 = np.asarray(w_fc1 * ln2_g[:, None] * WS, F8NP)
    fc1_bias = (ln2_b @ w_fc1 + np.asarray(b_fc1, np.float32)).astype(np.float32)
    w_fc28 = np.asarray(w_fc2 * WS, F8NP)

    shared = {
        "w_qkv8": w_qkv8,
        "w_proj8": w_proj8,
        "w_fc18": w_fc18,
        "w_fc28": w_fc28,
        "qkv_bias": qkv_bias,
        "fc1_bias": fc1_bias,
        "b_proj_eff": (np.asarray(b_proj, np.float32)
                       + qkv_bias[2 * C:] @ w_proj).astype(np.float32),
        "b_fc2": np.asarray(b_fc2, np.float32),
    }
    in_maps = []
    for c in range(8):
        b, hh = c // 2, c % 2
        xbv = np.ascontiguousarray(
            np.asarray(np.roll(x[b], -hh * NO, axis=0), BFNP))
        in_maps.append({"xb": xbv, **shared})

    if _NC_CACHE is None:
        _NC_CACHE = _build_bass()
    res = run_bass_kernel_spmd(_NC_CACHE, in_maps, core_ids=list(range(8)))

    outp = np.empty((B, N, C), np.float32)
    for c in range(8):
        b, hh = c // 2, c % 2
        outp[b, hh * NO:(hh + 1) * NO, :] = res.results[c]["out"]
    return outp


# revision 15
# speedup vs baseline: 2.0519x; 1.0100x over previous
"""Trainium2 Bass kernel for a dense transformer block (B=4, N=2048, C=768, H=12).

Sharding: 8 cores = 4 batches x 2 sequence halves (queries split; K/V duplicated
per batch pair, no collectives). Each core receives its batch rolled so its own
1024 query rows are rows 0:1023.

v2 dataflow (cost-model-driven):
- All projection/attention-value/MLP matmuls run in fp8e4m3 with DoubleRow perf
  mode (2 contraction k-tiles per instruction, 0.5 cycles/column = 4x fp32r).
  Weights are folded (LN gains), prescaled by 32 on the host, and cast to fp8;
  the 1/32 unscale + bias ride the PSUM->SBUF copies (or gelu's scale/bias).
- Scores stay bf16 (precision-sensitive); softmax exp runs on the Act engine in
  [128, 4, 512] PSUM tiles (2048-column calls) writing fp8 directly; the
  denominator rides the value matmul as a 65th ones-row of V.
- Per-query 1/den is applied via a PE ones-outer-product broadcast plus one DVE
  multiply per (head, chunk) - no transposes.
- LN uses Sqrt+DVE-reciprocal (no Ln/Exp table thrash); only 4 act-table loads.
- All DMA goes through SP HWDGE (Pool engine stays free for psum->sbuf copies).
"""

import numpy as np

B, N, C = 4, 2048, 768
H, DH = 12, 64
HID = 4 * C
SCALE = DH ** -0.5
EPS = 1e-5
WS = 32.0
IWS = 1.0 / WS

P = 128
CT = C // P          # 6
NT = N // P          # 16
NO = N // 2          # 1024 own rows
NOT_ = NO // P       # 8
HT = HID // P        # 24


def _build_bass():
    import concourse.bass as bass
    import concourse.tile as tile
    from concourse import bacc, mybir
    from concourse.masks import make_identity
    from concourse.alu_op_type import AluOpType as A

    F32 = mybir.dt.float32
    BF = mybir.dt.bfloat16
    F8 = mybir.dt.float8e4
    AF = mybir.ActivationFunctionType
    DR = mybir.MatmulPerfMode.DoubleRow

    nc = bacc.Bacc("TRN2", target_bir_lowering=False, num_swdge_queues=4)

    xb = nc.dram_tensor("xb", [N, C], BF, kind="ExternalInput")
    w_qkv8 = nc.dram_tensor("w_qkv8", [C, 3 * C], F8, kind="ExternalInput")
    w_proj8 = nc.dram_tensor("w_proj8", [DH, H * C], F8, kind="ExternalInput")
    w_fc18 = nc.dram_tensor("w_fc18", [C, HID], F8, kind="ExternalInput")
    w_fc28 = nc.dram_tensor("w_fc28", [HID, C], F8, kind="ExternalInput")
    qkv_bias = nc.dram_tensor("qkv_bias", [3 * C], F32, kind="ExternalInput")
    fc1_bias = nc.dram_tensor("fc1_bias", [HID], F32, kind="ExternalInput")
    b_proj_eff = nc.dram_tensor("b_proj_eff", [C], F32, kind="ExternalInput")
    b_fc2 = nc.dram_tensor("b_fc2", [C], F32, kind="ExternalInput")
    out = nc.dram_tensor("out", [NO, C], F32, kind="ExternalOutput")

    dma = nc.sync.dma_start

    with tile.TileContext(nc) as tc:
        consts = tc.alloc_tile_pool(name="consts", bufs=1)
        wpool = tc.alloc_tile_pool(name="wpool", bufs=1)
        big = tc.alloc_tile_pool(name="big", bufs=1)
        qkpool = tc.alloc_tile_pool(name="qkpool", bufs=2)
        work = tc.alloc_tile_pool(name="work", bufs=2)
        io = tc.alloc_tile_pool(name="io", bufs=6)
        estream = tc.alloc_tile_pool(name="estream", bufs=3)

        ident_bf = consts.tile([P, P], BF)
        make_identity(nc, ident_bf)
        ones_bf = consts.tile([P, DH], BF)
        nc.gpsimd.memset(ones_bf, 1.0)
        eps_t = consts.tile([P, 1], F32)
        nc.vector.memset(eps_t, EPS)
        qbT = consts.tile([P, 18], F32)
        dma(out=qbT, in_=qkv_bias[:].rearrange("(t p) -> p t", p=P))
        f1bT = consts.tile([P, HT], F32)
        dma(out=f1bT, in_=fc1_bias[:].rearrange("(t p) -> p t", p=P))
        bf2T = consts.tile([P, CT], F32)
        dma(out=bf2T, in_=b_fc2[:].rearrange("(t p) -> p t", p=P))
        bpT = consts.tile([P, CT], F32)
        dma(out=bpT, in_=b_proj_eff[:].rearrange("(t p) -> p t", p=P))

        Wqkv_s = wpool.tile([P, CT, 3 * C], F8)
        Wp_s = wpool.tile([DH, H, C], F8)
        Wfc1_s = wpool.tile([P, CT, HID], F8)
        Wfc2_s = wpool.tile([P, HT, C], F8)

        hT = big.tile([P, CT, N], F8)          # LN1(x)^T, fp8
        VW = 80  # V cols + ones + pad: dual-fp8 ldweights needs M % 16 == 0
        Vf8 = big.tile([P, NT, H, VW], F8)
        Yf8 = big.tile([DH, H, NO], F8)         # normalized y, proj rhs
        x2lnT = big.tile([P, CT, NO], F8)       # LN2(x2)^T
        x2 = big.tile([P, NOT_, C], BF)         # x + attn + b_proj

        nc.vector.memset(Vf8[:, :, :, DH:VW], 1.0)

        def ln_apply(src, dst_bf, csize):
            # dst = (src - mean) * rsqrt(var+eps); gains/biases are host-folded
            st = work.tile([P, 3, 6], F32, tag="ln_st", bufs=5)
            for s in range(3):
                nc.vector.bn_stats(out=st[:, s, :], in_=src[:, s * 256:(s + 1) * 256])
            mv = work.tile([P, 2], F32, tag="ln_mv", bufs=5)
            nc.vector.bn_aggr(out=mv, in_=st)
            sd = work.tile([P, 1], F32, tag="ln_sd", bufs=5)
            nc.scalar.activation(out=sd, in_=mv[:, 1:2], func=AF.Sqrt, bias=eps_t)
            r = work.tile([P, 1], F32, tag="ln_r", bufs=5)
            nc.vector.reciprocal(out=r, in_=sd)
            nc.gpsimd.tensor_scalar(out=dst_bf, in0=src, scalar1=mv[:, 0:1],
                                     scalar2=r, op0=A.subtract, op1=A.mult)

        # ---------------- Phase A: LN1 -> hT (fp8, transposed), V fused per tile
        # V weight columns load first (small); Q/K columns mid-loop.
        dma(out=Wqkv_s[:, :, 2 * C:],
            in_=w_qkv8[:, 2 * C:].rearrange("(t p) j -> p t j", p=P))
        with tc.tile_pool(name="ps_a", bufs=1, space="PSUM") as ps_a:
            for i in range(NT):
                x_t = io.tile([P, C], BF, tag="x")
                dma(out=x_t, in_=xb[i * P:(i + 1) * P, :])
                if i == 3:
                    dma(out=Wqkv_s[:, :, :2 * C],
                        in_=w_qkv8[:, :2 * C].rearrange("(t p) j -> p t j", p=P))
                z = work.tile([P, C], BF, tag="z", bufs=4)
                ln_apply(x_t, z, C)
                tp = ps_a.tile([P, C], BF, tag="tr", bufs=3)
                for t in range(CT):
                    nc.tensor.transpose(tp[:, t * P:(t + 1) * P],
                                        z[:, t * P:(t + 1) * P], ident_bf)
                nc.scalar.activation(
                    out=hT[:, :, i * P:(i + 1) * P],
                    in_=tp[:].rearrange("p (t n) -> p t n", t=CT), func=AF.Copy)
                # V for this key tile rides right behind its hT slice
                vps = ps_a.tile([P, C], F32, tag="v", bufs=2)
                for k2 in range(3):
                    for sl, c0 in ((512, 0), (256, 512)):
                        nc.tensor.matmul(
                            vps[:, c0:c0 + sl],
                            hT[:, 2 * k2:2 * k2 + 2, i * P:(i + 1) * P],
                            Wqkv_s[:, 2 * k2:2 * k2 + 2, 2 * C + c0:2 * C + c0 + sl],
                            start=(k2 == 0), stop=(k2 == 2), perf_mode=DR)
                if i % 2 == 0:
                    nc.vector.tensor_scalar(
                        out=Vf8[:, i, :, 0:DH],
                        in0=vps[:].rearrange("p (h d) -> p h d", h=H),
                        scalar1=IWS, scalar2=None, op0=A.mult)
                else:
                    nc.scalar.activation(
                        out=Vf8[:, i, :, 0:DH],
                        in_=vps[:].rearrange("p (h d) -> p h d", h=H),
                        func=AF.Copy, scale=IWS)
        dma(out=Wp_s, in_=w_proj8[:].rearrange("d (h j) -> d h j", h=H))
        dma(out=Wfc1_s, in_=w_fc18[:].rearrange("(t p) j -> p t j", p=P))
        dma(out=Wfc2_s, in_=w_fc28[:].rearrange("(t p) j -> p t j", p=P))

        qk_tiles = {}

        def qk_produce(hp, pool):
            # Q (own rows) and K (all rows) for head pair hp
            QT_s = qkpool.tile([P, NO], BF, tag="qt")
            for q2 in range(2):
                qps = pool.tile([P, 512], F32, tag="qk")
                for k2 in range(3):
                    nc.tensor.matmul(
                        qps,
                        Wqkv_s[:, 2 * k2:2 * k2 + 2, hp * P:(hp + 1) * P],
                        hT[:, 2 * k2:2 * k2 + 2, q2 * 512:(q2 + 1) * 512],
                        start=(k2 == 0), stop=(k2 == 2), perf_mode=DR)
                nc.vector.tensor_scalar(
                    out=QT_s[:, q2 * 512:(q2 + 1) * 512], in0=qps,
                    scalar1=IWS, scalar2=qbT[:, hp:hp + 1],
                    op0=A.mult, op1=A.add)
            KT_s = qkpool.tile([P, N], BF, tag="kt")
            for q4 in range(4):
                kps = pool.tile([P, 512], F32, tag="qk")
                for k2 in range(3):
                    nc.tensor.matmul(
                        kps,
                        Wqkv_s[:, 2 * k2:2 * k2 + 2, C + hp * P:C + (hp + 1) * P],
                        hT[:, 2 * k2:2 * k2 + 2, q4 * 512:(q4 + 1) * 512],
                        start=(k2 == 0), stop=(k2 == 2), perf_mode=DR)
                nc.vector.tensor_scalar(
                    out=KT_s[:, q4 * 512:(q4 + 1) * 512], in0=kps,
                    scalar1=IWS, scalar2=qbT[:, 6 + hp:7 + hp],
                    op0=A.mult, op1=A.add)
            qk_tiles[hp] = (QT_s, KT_s)

        # ---------------- Phase B: attention, head-sequential
        with tc.tile_pool(name="ps_b", bufs=1, space="PSUM") as ps_b:
            qk_produce(0, ps_b)
            for h in range(H):
                hp, sub = h // 2, h % 2
                base = sub * DH
                if sub == 0 and hp > 0:
                    qk_produce(hp, ps_b)
                QT_s, KT_s = qk_tiles[hp]
                for ch in range(2):
                    y = ps_b.tile([80, 512], F32, tag="y", bufs=2)
                    for g in range(8):
                        S = ps_b.tile([P, 2, 512], F32, tag="s", bufs=2)
                        for j in range(2):
                            m = 2 * g + j
                            nc.tensor.matmul(
                                S[:, j, :],
                                KT_s[base:base + DH, m * P:(m + 1) * P],
                                QT_s[base:base + DH, ch * 512:(ch + 1) * 512],
                                start=True, stop=True)
                        E8 = estream.tile([P, 2, 512], F8, tag="e")
                        nc.scalar.activation(out=E8, in_=S, func=AF.Exp, scale=SCALE)
                        nc.tensor.matmul(
                            y, Vf8[:, 2 * g:2 * g + 2, h, :], E8,
                            start=(g == 0), stop=(g == 7), perf_mode=DR)
                    # normalize: rinv broadcast via PE ones outer product, copy to
                    # SBUF (DVE pairs with one PSUM operand max), one multiply
                    rin = work.tile([P, 512], BF, tag="ri")
                    with nc.allow_low_precision(reason="bf16 softmax denom recip"):
                        nc.vector.reciprocal(out=rin[DH:DH + 1, :], in_=y[DH:DH + 1, :])
                    rb = ps_b.tile([DH, 512], F32, tag="rb")
                    nc.tensor.matmul(rb, ones_bf[DH:DH + 1, 0:DH],
                                     rin[DH:DH + 1, :], start=True, stop=True)
                    rbs = work.tile([DH, 512], BF, tag="rb")
                    nc.vector.tensor_copy(out=rbs, in_=rb)
                    nc.vector.tensor_tensor(
                        out=Yf8[:, h, ch * 512:(ch + 1) * 512],
                        in0=y[0:DH, :], in1=rbs, op=A.mult)

        # ---------------- Phase C: proj -> + x + b_proj -> x2
        with tc.tile_pool(name="ps_c", bufs=1, space="PSUM") as ps_c:
            for th in range(2):
                attnT = work.tile([P, CT, 512], BF, tag="at")
                for oc in range(CT):
                    pps = ps_c.tile([P, 512], F32, tag="p", bufs=2)
                    for j in range(CT):
                        nc.tensor.matmul(
                            pps, Wp_s[:, 2 * j:2 * j + 2, oc * P:(oc + 1) * P],
                            Yf8[:, 2 * j:2 * j + 2, th * 512:(th + 1) * 512],
                            start=(j == 0), stop=(j == CT - 1), perf_mode=DR)
                    nc.vector.tensor_scalar(out=attnT[:, oc, :], in0=pps,
                                            scalar1=IWS, scalar2=bpT[:, oc:oc + 1],
                                            op0=A.mult, op1=A.add)
                for i in range(4):
                    it = th * 4 + i
                    tpc = ps_c.tile([P, C], BF, tag="tr", bufs=2)
                    for t in range(CT):
                        nc.tensor.transpose(tpc[:, t * P:(t + 1) * P],
                                            attnT[:, t, i * P:(i + 1) * P], ident_bf)
                    x_t = io.tile([P, C], BF, tag="x")
                    dma(out=x_t, in_=xb[it * P:(it + 1) * P, :])
                    nc.vector.tensor_tensor(out=x2[:, it, :], in0=tpc, in1=x_t,
                                            op=A.add)

        # ---------------- Phase D: LN2 -> x2lnT
        with tc.tile_pool(name="ps_d", bufs=2, space="PSUM") as ps_d:
            for i in range(NOT_):
                z2 = work.tile([P, C], BF, tag="z", bufs=4)
                ln_apply(x2[:, i, :], z2, C)
                tpd = ps_d.tile([P, C], BF, tag="tr")
                for t in range(CT):
                    nc.tensor.transpose(tpd[:, t * P:(t + 1) * P],
                                        z2[:, t * P:(t + 1) * P], ident_bf)
                nc.scalar.activation(
                    out=x2lnT[:, :, i * P:(i + 1) * P],
                    in_=tpd[:].rearrange("p (t n) -> p t n", t=CT), func=AF.Copy)

        # ---------------- Phase E: MLP + residual -> out, per 512-token half
        with tc.tile_pool(name="ps_e", bufs=1, space="PSUM") as ps_e:
            for th in range(2):
                sl = slice(th * 512, (th + 1) * 512)
                GA8 = big.tile([P, HT, 512], F8, tag="ga", bufs=2)
                for hg in range(HT):
                    f1 = ps_e.tile([P, 512], F32, tag="f1", bufs=2)
                    for k2 in range(3):
                        nc.tensor.matmul(
                            f1, Wfc1_s[:, 2 * k2:2 * k2 + 2, hg * P:(hg + 1) * P],
                            x2lnT[:, 2 * k2:2 * k2 + 2, sl],
                            start=(k2 == 0), stop=(k2 == 2), perf_mode=DR)
                    nc.scalar.activation(out=GA8[:, hg, :], in_=f1, func=AF.Gelu,
                                         bias=f1bT[:, hg:hg + 1], scale=IWS)
                mlpT = work.tile([P, CT, 512], BF, tag="at")
                for oc in range(CT):
                    fa = ps_e.tile([P, 512], F32, tag="fa")
                    for j in range(12):
                        nc.tensor.matmul(
                            fa, Wfc2_s[:, 2 * j:2 * j + 2, oc * P:(oc + 1) * P],
                            GA8[:, 2 * j:2 * j + 2, :],
                            start=(j == 0), stop=(j == 11), perf_mode=DR)
                    nc.vector.tensor_scalar(out=mlpT[:, oc, :], in0=fa,
                                            scalar1=IWS, scalar2=bf2T[:, oc:oc + 1],
                                            op0=A.mult, op1=A.add)
                for i in range(4):
                    it = th * 4 + i
                    tpe = ps_e.tile([P, C], BF, tag="tr", bufs=2)
                    for t in range(CT):
                        nc.tensor.transpose(tpe[:, t * P:(t + 1) * P],
                                            mlpT[:, t, i * P:(i + 1) * P], ident_bf)
                    o_sb = work.tile([P, C], F32, tag="o")
                    nc.vector.tensor_tensor(out=o_sb, in0=tpe, in1=x2[:, it, :],
                                            op=A.add)
                    dma(out=out[it * P:(it + 1) * P, :], in_=o_sb)

        estream.release()
        io.release()
        work.release()
        qkpool.release()
        big.release()
        wpool.release()
        consts.release()

    nc.compile()
    return nc


_NC_CACHE = None


def kernel(x, ln1_g, ln1_b, w_qkv, w_proj, b_proj, ln2_g, ln2_b,
           w_fc1, b_fc1, w_fc2, b_fc2):
    global _NC_CACHE
    import ml_dtypes
    from concourse.bass_utils import run_bass_kernel_spmd

    F8NP = ml_dtypes.float8_e4m3
    BFNP = ml_dtypes.bfloat16

    x = np.asarray(x, np.float32)
    ln1_g = np.asarray(ln1_g, np.float32)
    ln1_b = np.asarray(ln1_b, np.float32)
    ln2_g = np.asarray(ln2_g, np.float32)
    ln2_b = np.asarray(ln2_b, np.float32)
    w_qkv = np.asarray(w_qkv, np.float32)
    w_proj = np.asarray(w_proj, np.float32)
    w_fc1 = np.asarray(w_fc1, np.float32)
    w_fc2 = np.asarray(w_fc2, np.float32)

    # host-side folding + fp8 prescaling
    w_qkv8 = np.asarray(w_qkv * ln1_g[:, None] * WS, F8NP)
    qkv_bias = (ln1_b @ w_qkv).astype(np.float32)
    # proj weights rearranged [DH, H, C] so head pairs share partitions 0:64
    w_proj8 = np.ascontiguousarray(
        np.asarray(w_proj * WS, F8NP).reshape(H, DH, C).transpose(1, 0, 2)
    ).reshape(DH, H * C)
    w_fc18 = np.asarray(w_fc1 * ln2_g[:, None] * WS, F8NP)
    fc1_bias = (ln2_b @ w_fc1 + np.asarray(b_fc1, np.float32)).astype(np.float32)
    w_fc28 = np.asarray(w_fc2 * WS, F8NP)

    shared = {
        "w_qkv8": w_qkv8,
        "w_proj8": w_proj8,
        "w_fc18": w_fc18,
        "w_fc28": w_fc28,
        "qkv_bias": qkv_bias,
        "fc1_bias": fc1_bias,
        "b_proj_eff": (np.asarray(b_proj, np.float32)
                       + qkv_bias[2 * C:] @ w_proj).astype(np.float32),
        "b_fc2": np.asarray(b_fc2, np.float32),
    }
    in_maps = []
    for c in range(8):
        b, hh = c // 2, c % 2
        xbv = np.ascontiguousarray(
            np.asarray(np.roll(x[b], -hh * NO, axis=0), BFNP))
        in_maps.append({"xb": xbv, **shared})

    if _NC_CACHE is None:
        _NC_CACHE = _build_bass()
    res = run_bass_kernel_spmd(_NC_CACHE, in_maps, core_ids=list(range(8)))

    outp = np.empty((B, N, C), np.float32)
    for c in range(8):
        b, hh = c // 2, c % 2
        outp[b, hh * NO:(hh + 1) * NO, :] = res.results[c]["out"]
    return outp
